# revision 35
# baseline (speedup 1.0000x reference)
"""Causal self-attention (B=2, S=2048, D=1024, H=16, Dh=64) on 8 NeuronCores.

Sharding: core c -> batch b = c//4, head-group g = c%4 (heads 4g..4g+3).
Each core computes QKV projection for its 4 heads, causal attention
(scores kept transposed: [k, q] layout so no on-chip transposes are
needed), and a partial output projection over its local head dims.
Host sums the 4 partials per batch and adds b_out.

v7: bf16 operands (fp32 PSUM accumulation), software-pipelined attention
units — each unit's second-head attn@V is deferred one slot so the
in-order PE queue always has the next unit's scores to run while the
scalar engine computes exp — plus DMA striped across both HWDGE queues
in need order, split-contraction first projection groups, V/projection/
output-projection work interleaved into the attention windows, and the
output DMA spread across the kernel.
"""

import numpy as np
from contextlib import ExitStack

B = 2
S = 2048
D = 1024
NH = 16
DH = 64
N_CORES = 8
HPC = 4            # heads per core
EL = HPC * DH      # 256 local head dims per core
KD = D // 128      # 8 contraction chunks for projections
KT = S // 128      # 16 key tiles

_NC = None
_last_in_maps = None


def _build_program():
    import concourse.mybir as mybir
    import concourse.tile as tile
    from concourse import bacc

    F32 = mybir.dt.float32
    BF16 = mybir.dt.bfloat16
    Exp = mybir.ActivationFunctionType.Exp

    nc = bacc.Bacc("TRN2", target_bir_lowering=False, debug=False,
                   num_devices=N_CORES)

    xt_d = nc.dram_tensor("xt", [D, S], BF16, kind="ExternalInput")
    wqk_d = nc.dram_tensor("wqk", [D, 2 * EL], BF16, kind="ExternalInput")
    wv_d = nc.dram_tensor("wv", [D, HPC * 65], BF16, kind="ExternalInput")
    bqk_d = nc.dram_tensor("bqk", [128, 4], F32, kind="ExternalInput")
    ones_d = nc.dram_tensor("ones", [1, 512], BF16, kind="ExternalInput")
    bv_d = nc.dram_tensor("bv", [1, HPC * 65], BF16, kind="ExternalInput")
    wo_d = nc.dram_tensor("wo", [EL, D], BF16, kind="ExternalInput")
    out_d = nc.dram_tensor("out", [S, D], BF16, kind="ExternalOutput")

    with nc.allow_low_precision(reason="bf16 matmul operands, fp32 accum"), \
         tile.TileContext(nc) as tc, ExitStack() as ctx:
        const = ctx.enter_context(tc.tile_pool(name="const", bufs=1))
        work = ctx.enter_context(tc.tile_pool(name="work", bufs=1))
        pin = ctx.enter_context(tc.tile_pool(name="pin", bufs=1))
        psS = ctx.enter_context(tc.tile_pool(name="psS", bufs=1, space="PSUM"))
        pb = ctx.enter_context(tc.tile_pool(name="pb", bufs=1, space="PSUM"))
        pp = ctx.enter_context(tc.tile_pool(name="pp", bufs=5))
        sm = ctx.enter_context(tc.tile_pool(name="sm", bufs=3))
        tn = ctx.enter_context(tc.tile_pool(name="tn", bufs=4))
        hb = ctx.enter_context(tc.tile_pool(name="hb", bufs=1))
        ob = ctx.enter_context(tc.tile_pool(name="ob", bufs=6))

        # ---------------- input DMAs ----------------
        # Striped across both HWDGE queues in compute-need order: the
        # first-half projection groups need xt[0..3]+wqk[0..3] first.
        xt_s = [pin.tile([128, S], BF16, tag=f"xt{k}", name=f"xt{k}") for k in range(KD)]
        wqk_s = [pin.tile([128, 2 * EL], BF16, tag=f"wqk{k}", name=f"wqk{k}") for k in range(KD)]
        wv_s = [pin.tile([128, HPC * 65], BF16, tag=f"wv{k}", name=f"wv{k}") for k in range(KD)]
        for k in range(0, KD, 2):
            nc.sync.dma_start(out=xt_s[k], in_=xt_d[128 * k:128 * (k + 1), :])
        for k in range(4):
            nc.scalar.dma_start(out=wqk_s[k], in_=wqk_d[128 * k:128 * (k + 1), :])
        for k in range(1, 4, 2):
            nc.scalar.dma_start(out=xt_s[k], in_=xt_d[128 * k:128 * (k + 1), :])
        for k in range(4, KD):
            nc.scalar.dma_start(out=wqk_s[k], in_=wqk_d[128 * k:128 * (k + 1), :])
        for k in range(5, KD, 2):
            nc.scalar.dma_start(out=xt_s[k], in_=xt_d[128 * k:128 * (k + 1), :])
        for k in range(KD):
            nc.scalar.dma_start(out=wv_s[k], in_=wv_d[128 * k:128 * (k + 1), :])
        wo_s = [const.tile([128, D], BF16, tag=f"wo{i}", name=f"wo{i}") for i in range(2)]
        for i in range(2):
            nc.scalar.dma_start(out=wo_s[i], in_=wo_d[128 * i:128 * (i + 1), :])
        bqk_s = const.tile([128, 4], F32, tag="bqk", name="bqk")
        nc.sync.dma_start(out=bqk_s, in_=bqk_d[:, :])
        bv_s = const.tile([1, HPC * 65], BF16, tag="bv", name="bv")
        nc.sync.dma_start(out=bv_s, in_=bv_d[:, :])
        ones_s = const.tile([1, 512], BF16, tag="ones", name="ones")
        nc.sync.dma_start(out=ones_s, in_=ones_d[:, :])
        # ones living on partition 64, for the denominator broadcast matmul
        ones64_s = const.tile([65, 64], BF16, tag="ones64", name="ones64")
        nc.sync.dma_start(out=ones64_s[64:65, :], in_=ones_d[0:1, 0:64])
        # zero weights for harmless PE keep-warm filler matmuls
        zeros_s = const.tile([128, 65], BF16, tag="zeros", name="zeros")
        nc.gpsimd.memset(zeros_s[:, :], 0.0)

        # ---------------- persistent SBUF tensors ----------------
        # qkT: e-tiles 0,1 = Q (head pairs 0,1), 2,3 = K
        qk_s = [work.tile([128, S], BF16, tag=f"qk{e}", name=f"qk{e}") for e in range(4)]
        # V augmented: per key-tile [128, 4*65]; col 64 of each head = 1.0
        vaug_s = [work.tile([128, HPC * 65], BF16, tag=f"va{t}", name=f"va{t}") for t in range(KT)]
        # normalized attn output, transposed: [d_local, s]
        attnT_s = [work.tile([128, S], BF16, tag=f"at{d}", name=f"at{d}") for d in range(2)]

        # ---------------- emission helpers ----------------
        nps = [0]

        def stag():
            nps[0] += 1
            return f"s{nps[0] % 2}"

        h_sb = {}

        def emit_qk_halfA(e, sc):
            """First half of the contraction (k 0..3) + bias, parked in SBUF."""
            ps = psS.tile([128, 512], F32, tag=stag(), name="psA")
            for k in range(4):
                nc.tensor.matmul(
                    ps,
                    lhsT=wqk_s[k][:, 128 * e:128 * (e + 1)],
                    rhs=xt_s[k][:, 512 * sc:512 * (sc + 1)],
                    start=(k == 0), stop=(k == 3))
            h = hb.tile([128, 512], F32, tag=f"h{e}{sc}", name=f"h{e}{sc}")
            nc.vector.tensor_scalar_add(out=h, in0=ps,
                                        scalar1=bqk_s[:, e:e + 1])
            h_sb[(e, sc)] = h

        def emit_qk_halfB(e, sc):
            """Second half (k 4..7); combined with the parked first half."""
            ps = psS.tile([128, 512], F32, tag=stag(), name="psB")
            for k in range(4, KD):
                nc.tensor.matmul(
                    ps,
                    lhsT=wqk_s[k][:, 128 * e:128 * (e + 1)],
                    rhs=xt_s[k][:, 512 * sc:512 * (sc + 1)],
                    start=(k == 4), stop=(k == KD - 1))
            nc.vector.tensor_add(
                out=qk_s[e][:, 512 * sc:512 * (sc + 1)],
                in0=ps, in1=h_sb[(e, sc)])

        def emit_qk_single(e, sc):
            ps = psS.tile([128, 512], F32, tag=stag(), name="psQ")
            for k in range(KD):
                nc.tensor.matmul(
                    ps,
                    lhsT=wqk_s[k][:, 128 * e:128 * (e + 1)],
                    rhs=xt_s[k][:, 512 * sc:512 * (sc + 1)],
                    start=(k == 0), stop=(k == KD - 1))
            nc.vector.tensor_scalar_add(
                out=qk_s[e][:, 512 * sc:512 * (sc + 1)], in0=ps,
                scalar1=bqk_s[:, e:e + 1])

        def emit_v_group(st):
            ps = psS.tile([128, HPC * 65], F32, tag=stag(), name="psV")
            for k in range(KD):
                nc.tensor.matmul(
                    ps,
                    lhsT=xt_s[k][:, 128 * st:128 * (st + 1)],
                    rhs=wv_s[k],
                    start=(k == 0), stop=False)
            nc.tensor.matmul(ps, lhsT=ones_s[0:1, 0:128], rhs=bv_s,
                             start=False, stop=True)
            nc.vector.tensor_copy(vaug_s[st], ps)

        # attention state per (hp, qh): dict (h2, jq) -> psum tile
        OT_TAGS = {(0, 0): "o00", (1, 0): "o10", (0, 1): "o01", (1, 1): "o11"}

        def emit_ot_alloc(qh):
            return {
                (h2, jq): pb.tile([65, 512], F32, tag=OT_TAGS[(h2, jq)],
                                  name=f"ot{h2}{jq}")
                for h2 in range(2) for jq in range(2)
            }

        def emit_normalize_cols(hp, h2, t, jq_g, c0, c1):
            """Normalize ot columns [c0:c1) into attnT."""
            w = c1 - c0
            den = sm.tile([65, 512], BF16, tag="den", name="den")
            nc.vector.tensor_copy(den[64:65, 0:w], t[64:65, c0:c1])
            rb_ps = psS.tile([64, 512], F32, tag=stag(), name="rb")
            nc.tensor.matmul(rb_ps[:, 0:w], lhsT=ones64_s[64:65, :],
                             rhs=den[64:65, 0:w], start=True, stop=True)
            rb_sb = sm.tile([64, 512], F32, tag="rbs", name="rbs")
            nc.vector.reciprocal_approx_fast(out=rb_sb[:, 0:w],
                                             in_=rb_ps[:, 0:w])
            a0 = 512 * jq_g + c0
            if h2 == 0:
                nc.vector.tensor_mul(
                    out=attnT_s[hp][0:64, a0:a0 + w],
                    in0=t[0:64, c0:c1], in1=rb_sb[:, 0:w])
            else:
                t_n = tn.tile([64, 512], BF16, tag="tn", name="tn")
                nc.vector.tensor_mul(out=t_n[:, 0:w], in0=t[0:64, c0:c1],
                                     in1=rb_sb[:, 0:w])
                nc.sync.dma_start(
                    out=attnT_s[hp][64:128, a0:a0 + w],
                    in_=t_n[:, 0:w])

        def emit_normalize(hp, qh, h2, jq, ot):
            emit_normalize_cols(hp, h2, ot[(h2, jq)], 2 * qh + jq, 0, 512)

        def emit_scores_pair(hp, qh, ki):
            """Scores for both heads of the pair, interleaved h0/h1 per
            q-chunk so the 64-row matmuls overlap on row groups 0/64."""
            qt = qk_s[hp]
            kt_ = qk_s[2 + hp]
            s_ps = [psS.tile([128, 1024], F32, tag=f"s{h2}", name=f"s{h2}")
                    for h2 in range(2)]
            for qq in range(2):
                q0 = 1024 * qh + 512 * qq
                if q0 + 512 <= 128 * ki:
                    continue
                for h2 in range(2):
                    pbase = 64 * h2
                    nc.tensor.matmul(
                        s_ps[h2][:, 512 * qq:512 * (qq + 1)],
                        lhsT=kt_[pbase:pbase + 64, 128 * ki:128 * (ki + 1)],
                        rhs=qt[pbase:pbase + 64, q0:q0 + 512],
                        start=True, stop=True)
            return s_ps

        def emit_exp(qh, ki, s_ps):
            off = max(0, 128 * ki - 1024 * qh)
            p_t = pp.tile([128, 1024], BF16, tag="p", name="pt")
            nc.scalar.activation(
                out=p_t[:, off:1024], in_=s_ps[:, off:1024],
                func=Exp, scale=0.125)
            if 128 * ki >= 1024 * qh:
                dof = 128 * ki - 1024 * qh
                nc.gpsimd.affine_select(
                    out=p_t[:, dof:dof + 128],
                    in_=p_t[:, dof:dof + 128],
                    compare_op=mybir.AluOpType.is_ge, fill=0.0,
                    base=0, pattern=[[1, 128]], channel_multiplier=-1)
            return p_t

        def emit_attnv(hp, qh, ki, h2, p_t, ot):
            h = 2 * hp + h2
            for jq in range(2):
                jq_g = 2 * qh + jq
                if ki > 4 * jq_g + 3:
                    continue
                q0g = max(512 * jq_g, 128 * ki)
                nc.tensor.matmul(
                    ot[(h2, jq)][:, q0g - 512 * jq_g:512],
                    lhsT=vaug_s[ki][:, 65 * h:65 * h + 65],
                    rhs=p_t[:, q0g - 1024 * qh:
                                512 * (jq_g + 1) - 1024 * qh],
                    start=(ki == 0), stop=(ki == 4 * jq_g + 3),
                    skip_group_check=True)
            for jq in range(2):
                jq_g = 2 * qh + jq
                if jq_g == 3:
                    # the final chunk's column blocks finish progressively:
                    # block m is final after key-tile 12+m — normalize
                    # eagerly to shorten the serial kernel tail
                    if 12 <= ki <= 15:
                        m = ki - 12
                        emit_normalize_cols(hp, h2, ot[(h2, jq)], 3,
                                            128 * m, 128 * (m + 1))
                elif ki == 4 * jq_g + 3:
                    emit_normalize(hp, qh, h2, jq, ot)

        def emit_section(hp, qh, nki, ot, inserts):
            """Software-pipelined attention section: the h2=1 attn@V of
            each unit is deferred one slot so the PE has the next unit's
            scores available while ACT runs exp."""
            pend = None
            for ki in range(nki):
                ins = inserts.get(ki)
                if ins:
                    for f in ins:
                        f()
                s01 = emit_scores_pair(hp, qh, ki)
                if pend is not None:
                    emit_attnv(hp, qh, pend[0], 1, pend[1], ot)
                p0 = emit_exp(qh, ki, s01[0])
                emit_attnv(hp, qh, ki, 0, p0, ot)
                p1 = emit_exp(qh, ki, s01[1])
                pend = (ki, p1)
            emit_attnv(hp, qh, pend[0], 1, pend[1], ot)

        def emit_filler(ot):
            """Zero-weight accumulates into the live jq=1 groups: keep the
            PE issue stream dense (HAM stays warm) without changing math."""
            for h2 in range(2):
                nc.tensor.matmul(
                    ot[(h2, 1)][:, 0:512],
                    lhsT=zeros_s[:, 0:65],
                    rhs=qk_s[h2][:, 0:512],
                    start=False, stop=False,
                    skip_group_check=True)

        def emit_c_group(st, ec, tag):
            ps = pb.tile([128, 512], F32, tag=tag, name="psC")
            for dl in range(2):
                nc.tensor.matmul(
                    ps,
                    lhsT=attnT_s[dl][:, 128 * st:128 * (st + 1)],
                    rhs=wo_s[dl][:, 512 * ec:512 * (ec + 1)],
                    start=(dl == 0), stop=(dl == 1))
            o_t = ob.tile([128, 512], BF16, tag="ob", name="ob")
            nc.vector.tensor_copy(o_t, ps)
            nc.sync.dma_start(
                out=out_d[128 * st:128 * (st + 1),
                          512 * ec:512 * (ec + 1)],
                in_=o_t)

        def emit_c_chunk(qh, jq):
            """Output projection for the 4 seq tiles of chunk jq_g=2qh+jq."""
            jq_g = 2 * qh + jq
            tags = [OT_TAGS[(0, jq)], OT_TAGS[(1, jq)]]
            i = 0
            for st in range(4 * jq_g, 4 * jq_g + 4):
                for ec in range(2):
                    emit_c_group(st, ec, tags[i % 2])
                    i += 1

        # ---------------- emission schedule ----------------
        # 1) K/Q head pair 0: all first-half groups (need only xt[0..3]),
        #    then all second-half groups.
        for e in (2, 0):
            for sc in range(4):
                emit_qk_halfA(e, sc)
        for e in (2, 0):
            for sc in range(4):
                emit_qk_halfB(e, sc)

        # 2) attention (hp=0, qh=0) with V st 0..7 and the head-pair-1
        #    K/Q groups needed early by section 4 interleaved
        ot00 = emit_ot_alloc(0)
        ins2 = {ki: [lambda st=ki: emit_v_group(st)] for ki in range(8)}
        ins2[4].append(lambda: emit_qk_single(3, 0))
        ins2[5].append(lambda: emit_qk_single(1, 0))
        ins2[6].append(lambda: emit_qk_single(1, 1))
        ins2[7].append(lambda: emit_qk_single(3, 1))
        emit_section(0, 0, 8, ot00, ins2)

        # 4) attention (hp=1, qh=0) with V st 8..13 interleaved and
        #    C(qh0, jq0) in the tail slots (tags free after slot-3/4
        #    normalizes)
        ot10 = emit_ot_alloc(0)
        ins4 = {ki: [lambda st=8 + ki: emit_v_group(st)] for ki in range(6)}
        cg0 = [(st, ec) for st in range(4) for ec in range(2)]
        ins4[5] = ins4[5] + [
            lambda st=st, ec=ec: emit_c_group(st, ec, OT_TAGS[((st + ec) % 2, 0)])
            for st, ec in cg0[0:2]]
        ins4[6] = [
            lambda st=st, ec=ec: emit_c_group(st, ec, OT_TAGS[((st + ec) % 2, 0)])
            for st, ec in cg0[2:5]]
        ins4[7] = [
            lambda st=st, ec=ec: emit_c_group(st, ec, OT_TAGS[((st + ec) % 2, 0)])
            for st, ec in cg0[5:8]]
        emit_section(1, 0, 8, ot10, ins4)

        # C(qh0, jq1) while its pb tags are free
        emit_c_chunk(0, 1)

        # 5) attention (hp=0, qh=1) with V st 14,15 and remaining K/Q
        #    pair-1 groups interleaved; fillers keep the PE dense
        ot01 = emit_ot_alloc(1)
        ins5 = {
            0: [lambda: emit_v_group(14)],
            1: [lambda: emit_v_group(15)],
            2: [lambda: emit_qk_single(1, 2)],
            3: [lambda: emit_qk_single(1, 3)],
            4: [lambda: emit_qk_single(3, 2)],
            5: [lambda: emit_qk_single(3, 3)],
        }
        for ki in range(6, 16):
            ins5[ki] = [lambda: emit_filler(ot01),
                        lambda: emit_filler(ot01)]
        emit_section(0, 1, 16, ot01, ins5)

        # 6) attention (hp=1, qh=1); C groups flow into the tail slots as
        #    soon as both head pairs have normalized each 128-q block
        def ctag(st, ec):
            return OT_TAGS[((st + ec) % 2, 0)]

        ot11 = emit_ot_alloc(1)
        ins6 = {}
        for ki in range(1, 13):
            ins6[ki] = [lambda: emit_filler(ot11),
                        lambda: emit_filler(ot11)]
        tailc = {
            13: [(8, 0), (8, 1), (9, 0)],
            14: [(9, 1), (10, 0), (10, 1), (12, 0), (12, 1)],
            15: [(11, 0), (11, 1), (13, 0), (13, 1)],
        }
        for ki, groups in tailc.items():
            ins6[ki] = [
                lambda st=st, ec=ec: emit_c_group(st, ec, ctag(st, ec))
                for st, ec in groups]
        emit_section(1, 1, 16, ot11, ins6)
        for st, ec in ((14, 0), (14, 1), (15, 0), (15, 1)):
            emit_c_group(st, ec, ctag(st, ec))

    nc.compile()
    return nc


def _get_program():
    global _NC
    if _NC is None:
        _NC = _build_program()
    return _NC


def kernel(x, w_qkv, b_qkv, w_out, b_out):
    import ml_dtypes
    from concourse.bass_utils import run_bass_kernel_spmd

    BF = ml_dtypes.bfloat16
    x = np.asarray(x, dtype=np.float32)
    w_qkv = np.asarray(w_qkv, dtype=np.float32)
    b_qkv = np.asarray(b_qkv, dtype=np.float32)
    w_out = np.asarray(w_out, dtype=np.float32)
    b_out = np.asarray(b_out, dtype=np.float32)

    nc = _get_program()

    in_maps = []
    for c in range(N_CORES):
        b = c // 4
        g = c % 4
        hs = slice(g * EL, (g + 1) * EL)
        wq = w_qkv[0 * D:1 * D][hs]          # [256, 1024]
        wk = w_qkv[1 * D:2 * D][hs]
        wv = w_qkv[2 * D:3 * D][hs]
        bq = b_qkv[0 * D:1 * D][hs]
        bk = b_qkv[1 * D:2 * D][hs]
        bv = b_qkv[2 * D:3 * D][hs]
        bqk = np.concatenate([bq, bk])       # [512]
        wvx = np.zeros((D, HPC * 65), dtype=np.float32)
        bvx = np.zeros((1, HPC * 65), dtype=np.float32)
        for h in range(HPC):
            wvx[:, 65 * h:65 * h + 64] = wv[h * DH:(h + 1) * DH].T
            bvx[0, 65 * h:65 * h + 64] = bv[h * DH:(h + 1) * DH]
            bvx[0, 65 * h + 64] = 1.0
        in_maps.append({
            "xt": np.ascontiguousarray(x[b].T).astype(BF),             # [1024, 2048]
            "wqk": np.ascontiguousarray(np.concatenate([wq, wk]).T).astype(BF),
            "wv": wvx.astype(BF),                                      # [1024, 260]
            "bqk": np.ascontiguousarray(bqk.reshape(4, 128).T),        # [128, 4] f32
            "bv": bvx.astype(BF),                                      # [1, 260]
            "ones": np.ones((1, 512), dtype=BF),
            "wo": np.ascontiguousarray(w_out[:, hs].T).astype(BF),     # [256, 1024]
        })

    global _last_in_maps
    _last_in_maps = in_maps
    res = run_bass_kernel_spmd(nc, in_maps, list(range(N_CORES)))

    out = np.empty((B, S, D), dtype=np.float32)
    for b in range(B):
        acc = res.results[4 * b]["out"].astype(np.float32)
        for j in range(1, 4):
            acc = acc + res.results[4 * b + j]["out"].astype(np.float32)
        out[b] = acc + b_out[None, :]
    return out


# revision 38
# speedup vs baseline: 1.0090x; 1.0090x over previous
"""Causal self-attention (B=2, S=2048, D=1024, H=16, Dh=64) on 8 NeuronCores.

Sharding: core c -> batch b = c//4, head-group g = c%4 (heads 4g..4g+3).
Each core computes QKV projection for its 4 heads, causal attention
(scores kept transposed: [k, q] layout so no on-chip transposes are
needed), and a partial output projection over its local head dims.
Host sums the 4 partials per batch and adds b_out.

v7: bf16 operands (fp32 PSUM accumulation), software-pipelined attention
units — each unit's second-head attn@V is deferred one slot so the
in-order PE queue always has the next unit's scores to run while the
scalar engine computes exp — plus DMA striped across both HWDGE queues
in need order, split-contraction first projection groups, V/projection/
output-projection work interleaved into the attention windows, and the
output DMA spread across the kernel.
"""

import numpy as np
from contextlib import ExitStack

B = 2
S = 2048
D = 1024
NH = 16
DH = 64
N_CORES = 8
HPC = 4            # heads per core
EL = HPC * DH      # 256 local head dims per core
KD = D // 128      # 8 contraction chunks for projections
KT = S // 128      # 16 key tiles

_NC = None
_last_in_maps = None


def _build_program():
    import concourse.mybir as mybir
    import concourse.tile as tile
    from concourse import bacc

    F32 = mybir.dt.float32
    BF16 = mybir.dt.bfloat16
    Exp = mybir.ActivationFunctionType.Exp

    nc = bacc.Bacc("TRN2", target_bir_lowering=False, debug=False,
                   num_devices=N_CORES)

    xt_d = nc.dram_tensor("xt", [D, S], BF16, kind="ExternalInput")
    wqk_d = nc.dram_tensor("wqk", [D, 2 * EL], BF16, kind="ExternalInput")
    wv_d = nc.dram_tensor("wv", [D, HPC * 65], BF16, kind="ExternalInput")
    bqk_d = nc.dram_tensor("bqk", [128, 4], F32, kind="ExternalInput")
    ones_d = nc.dram_tensor("ones", [1, 512], BF16, kind="ExternalInput")
    bv_d = nc.dram_tensor("bv", [1, HPC * 65], BF16, kind="ExternalInput")
    wo_d = nc.dram_tensor("wo", [EL, D], BF16, kind="ExternalInput")
    out_d = nc.dram_tensor("out", [S, D], BF16, kind="ExternalOutput")

    with nc.allow_low_precision(reason="bf16 matmul operands, fp32 accum"), \
         tile.TileContext(nc) as tc, ExitStack() as ctx:
        const = ctx.enter_context(tc.tile_pool(name="const", bufs=1))
        work = ctx.enter_context(tc.tile_pool(name="work", bufs=1))
        pin = ctx.enter_context(tc.tile_pool(name="pin", bufs=1))
        psS = ctx.enter_context(tc.tile_pool(name="psS", bufs=1, space="PSUM"))
        pb = ctx.enter_context(tc.tile_pool(name="pb", bufs=1, space="PSUM"))
        pp = ctx.enter_context(tc.tile_pool(name="pp", bufs=4))
        sm = ctx.enter_context(tc.tile_pool(name="sm", bufs=2))
        tn = ctx.enter_context(tc.tile_pool(name="tn", bufs=3))
        hb = ctx.enter_context(tc.tile_pool(name="hb", bufs=1))
        ob = ctx.enter_context(tc.tile_pool(name="ob", bufs=4))

        # ---------------- input DMAs ----------------
        # Striped across both HWDGE queues in compute-need order: the
        # first-half projection groups need xt[0..3]+wqk[0..3] first.
        xt_s = [pin.tile([128, S], BF16, tag=f"xt{k}", name=f"xt{k}") for k in range(KD)]
        wqk_s = [pin.tile([128, 2 * EL], BF16, tag=f"wqk{k}", name=f"wqk{k}") for k in range(KD)]
        wv_s = [pin.tile([128, HPC * 65], BF16, tag=f"wv{k}", name=f"wv{k}") for k in range(KD)]
        for k in range(0, KD, 2):
            nc.sync.dma_start(out=xt_s[k], in_=xt_d[128 * k:128 * (k + 1), :])
        for k in range(4):
            nc.scalar.dma_start(out=wqk_s[k], in_=wqk_d[128 * k:128 * (k + 1), :])
        for k in range(1, 4, 2):
            nc.scalar.dma_start(out=xt_s[k], in_=xt_d[128 * k:128 * (k + 1), :])
        for k in range(4, KD):
            nc.scalar.dma_start(out=wqk_s[k], in_=wqk_d[128 * k:128 * (k + 1), :])
        for k in range(5, KD, 2):
            nc.scalar.dma_start(out=xt_s[k], in_=xt_d[128 * k:128 * (k + 1), :])
        for k in range(KD):
            nc.scalar.dma_start(out=wv_s[k], in_=wv_d[128 * k:128 * (k + 1), :])
        wo_s = [const.tile([128, D], BF16, tag=f"wo{i}", name=f"wo{i}") for i in range(2)]
        for i in range(2):
            nc.scalar.dma_start(out=wo_s[i], in_=wo_d[128 * i:128 * (i + 1), :])
        bqk_s = const.tile([128, 4], F32, tag="bqk", name="bqk")
        nc.sync.dma_start(out=bqk_s, in_=bqk_d[:, :])
        bv_s = const.tile([1, HPC * 65], BF16, tag="bv", name="bv")
        nc.sync.dma_start(out=bv_s, in_=bv_d[:, :])
        ones_s = const.tile([1, 512], BF16, tag="ones", name="ones")
        nc.sync.dma_start(out=ones_s, in_=ones_d[:, :])
        # ones living on partition 64, for the denominator broadcast matmul
        ones64_s = const.tile([65, 64], BF16, tag="ones64", name="ones64")
        nc.sync.dma_start(out=ones64_s[64:65, :], in_=ones_d[0:1, 0:64])
        # zero weights for harmless PE keep-warm filler matmuls
        zeros_s = const.tile([128, 65], BF16, tag="zeros", name="zeros")
        nc.gpsimd.memset(zeros_s[:, :], 0.0)

        # ---------------- persistent SBUF tensors ----------------
        # qkT: e-tiles 0,1 = Q (head pairs 0,1), 2,3 = K
        qk_s = [work.tile([128, S], BF16, tag=f"qk{e}", name=f"qk{e}") for e in range(4)]
        # V augmented: per key-tile [128, 4*65]; col 64 of each head = 1.0
        vaug_s = [work.tile([128, HPC * 65], BF16, tag=f"va{t}", name=f"va{t}") for t in range(KT)]
        # normalized attn output, transposed: [d_local, s]
        attnT_s = [work.tile([128, S], BF16, tag=f"at{d}", name=f"at{d}") for d in range(2)]

        # ---------------- emission helpers ----------------
        nps = [0]

        def stag():
            nps[0] += 1
            return f"s{nps[0] % 2}"

        h_sb = {}

        def emit_qk_halfA(e, sc):
            """First half of the contraction (k 0..3) + bias, parked in SBUF."""
            ps = psS.tile([128, 512], F32, tag=stag(), name="psA")
            for k in range(4):
                nc.tensor.matmul(
                    ps,
                    lhsT=wqk_s[k][:, 128 * e:128 * (e + 1)],
                    rhs=xt_s[k][:, 512 * sc:512 * (sc + 1)],
                    start=(k == 0), stop=(k == 3))
            h = hb.tile([128, 512], F32, tag=f"h{e}{sc}", name=f"h{e}{sc}")
            nc.vector.tensor_scalar_add(out=h, in0=ps,
                                        scalar1=bqk_s[:, e:e + 1])
            h_sb[(e, sc)] = h

        def emit_qk_halfB(e, sc):
            """Second half (k 4..7); combined with the parked first half."""
            ps = psS.tile([128, 512], F32, tag=stag(), name="psB")
            for k in range(4, KD):
                nc.tensor.matmul(
                    ps,
                    lhsT=wqk_s[k][:, 128 * e:128 * (e + 1)],
                    rhs=xt_s[k][:, 512 * sc:512 * (sc + 1)],
                    start=(k == 4), stop=(k == KD - 1))
            nc.vector.tensor_add(
                out=qk_s[e][:, 512 * sc:512 * (sc + 1)],
                in0=ps, in1=h_sb[(e, sc)])

        def emit_qk_single(e, sc):
            ps = psS.tile([128, 512], F32, tag=stag(), name="psQ")
            for k in range(KD):
                nc.tensor.matmul(
                    ps,
                    lhsT=wqk_s[k][:, 128 * e:128 * (e + 1)],
                    rhs=xt_s[k][:, 512 * sc:512 * (sc + 1)],
                    start=(k == 0), stop=(k == KD - 1))
            nc.vector.tensor_scalar_add(
                out=qk_s[e][:, 512 * sc:512 * (sc + 1)], in0=ps,
                scalar1=bqk_s[:, e:e + 1])

        def emit_v_group(st):
            ps = psS.tile([128, HPC * 65], F32, tag=stag(), name="psV")
            for k in range(KD):
                nc.tensor.matmul(
                    ps,
                    lhsT=xt_s[k][:, 128 * st:128 * (st + 1)],
                    rhs=wv_s[k],
                    start=(k == 0), stop=False)
            nc.tensor.matmul(ps, lhsT=ones_s[0:1, 0:128], rhs=bv_s,
                             start=False, stop=True)
            nc.vector.tensor_copy(vaug_s[st], ps)

        # attention state per (hp, qh): dict (h2, jq) -> psum tile
        OT_TAGS = {(0, 0): "o00", (1, 0): "o10", (0, 1): "o01", (1, 1): "o11"}

        def emit_ot_alloc(qh):
            return {
                (h2, jq): pb.tile([65, 512], F32, tag=OT_TAGS[(h2, jq)],
                                  name=f"ot{h2}{jq}")
                for h2 in range(2) for jq in range(2)
            }

        def emit_normalize_cols(hp, h2, t, jq_g, c0, c1):
            """Normalize ot columns [c0:c1) into attnT."""
            w = c1 - c0
            den = sm.tile([65, 512], BF16, tag="den", name="den")
            nc.vector.tensor_copy(den[64:65, 0:w], t[64:65, c0:c1])
            rb_ps = psS.tile([64, 512], F32, tag=stag(), name="rb")
            nc.tensor.matmul(rb_ps[:, 0:w], lhsT=ones64_s[64:65, :],
                             rhs=den[64:65, 0:w], start=True, stop=True)
            rb_sb = sm.tile([64, 512], F32, tag="rbs", name="rbs")
            nc.vector.reciprocal_approx_fast(out=rb_sb[:, 0:w],
                                             in_=rb_ps[:, 0:w])
            a0 = 512 * jq_g + c0
            if h2 == 0:
                nc.vector.tensor_mul(
                    out=attnT_s[hp][0:64, a0:a0 + w],
                    in0=t[0:64, c0:c1], in1=rb_sb[:, 0:w])
            else:
                t_n = tn.tile([64, 512], BF16, tag="tn", name="tn")
                nc.vector.tensor_mul(out=t_n[:, 0:w], in0=t[0:64, c0:c1],
                                     in1=rb_sb[:, 0:w])
                nc.sync.dma_start(
                    out=attnT_s[hp][64:128, a0:a0 + w],
                    in_=t_n[:, 0:w])

        def emit_normalize(hp, qh, h2, jq, ot):
            emit_normalize_cols(hp, h2, ot[(h2, jq)], 2 * qh + jq, 0, 512)

        def emit_scores_pair(hp, qh, ki):
            """Scores for both heads of the pair, interleaved h0/h1 per
            q-chunk so the 64-row matmuls overlap on row groups 0/64."""
            qt = qk_s[hp]
            kt_ = qk_s[2 + hp]
            s_ps = [psS.tile([128, 1024], F32, tag=f"s{h2}", name=f"s{h2}")
                    for h2 in range(2)]
            for qq in range(2):
                q0 = 1024 * qh + 512 * qq
                if q0 + 512 <= 128 * ki:
                    continue
                for h2 in range(2):
                    pbase = 64 * h2
                    nc.tensor.matmul(
                        s_ps[h2][:, 512 * qq:512 * (qq + 1)],
                        lhsT=kt_[pbase:pbase + 64, 128 * ki:128 * (ki + 1)],
                        rhs=qt[pbase:pbase + 64, q0:q0 + 512],
                        start=True, stop=True)
            return s_ps

        def emit_exp(qh, ki, s_ps):
            off = max(0, 128 * ki - 1024 * qh)
            p_t = pp.tile([128, 1024], BF16, tag="p", name="pt")
            nc.scalar.activation(
                out=p_t[:, off:1024], in_=s_ps[:, off:1024],
                func=Exp, scale=0.125)
            if 128 * ki >= 1024 * qh:
                dof = 128 * ki - 1024 * qh
                nc.gpsimd.affine_select(
                    out=p_t[:, dof:dof + 128],
                    in_=p_t[:, dof:dof + 128],
                    compare_op=mybir.AluOpType.is_ge, fill=0.0,
                    base=0, pattern=[[1, 128]], channel_multiplier=-1)
            return p_t

        def emit_attnv(hp, qh, ki, h2, p_t, ot):
            h = 2 * hp + h2
            for jq in range(2):
                jq_g = 2 * qh + jq
                if ki > 4 * jq_g + 3:
                    continue
                q0g = max(512 * jq_g, 128 * ki)
                nc.tensor.matmul(
                    ot[(h2, jq)][:, q0g - 512 * jq_g:512],
                    lhsT=vaug_s[ki][:, 65 * h:65 * h + 65],
                    rhs=p_t[:, q0g - 1024 * qh:
                                512 * (jq_g + 1) - 1024 * qh],
                    start=(ki == 0), stop=(ki == 4 * jq_g + 3),
                    skip_group_check=True)
            for jq in range(2):
                if ki == 4 * (2 * qh + jq) + 3:
                    emit_normalize(hp, qh, h2, jq, ot)

        def emit_section(hp, qh, nki, ot, inserts):
            """Software-pipelined attention section: the h2=1 attn@V of
            each unit is deferred one slot so the PE has the next unit's
            scores available while ACT runs exp."""
            pend = None
            for ki in range(nki):
                ins = inserts.get(ki)
                if ins:
                    for f in ins:
                        f()
                s01 = emit_scores_pair(hp, qh, ki)
                if pend is not None:
                    emit_attnv(hp, qh, pend[0], 1, pend[1], ot)
                p0 = emit_exp(qh, ki, s01[0])
                emit_attnv(hp, qh, ki, 0, p0, ot)
                p1 = emit_exp(qh, ki, s01[1])
                pend = (ki, p1)
            emit_attnv(hp, qh, pend[0], 1, pend[1], ot)

        def emit_filler(ot):
            """Zero-weight accumulates into the live jq=1 groups: keep the
            PE issue stream dense (HAM stays warm) without changing math."""
            for h2 in range(2):
                nc.tensor.matmul(
                    ot[(h2, 1)][:, 0:512],
                    lhsT=zeros_s[:, 0:65],
                    rhs=qk_s[h2][:, 0:512],
                    start=False, stop=False,
                    skip_group_check=True)

        def emit_c_group(st, ec, tag):
            ps = pb.tile([128, 512], F32, tag=tag, name="psC")
            for dl in range(2):
                nc.tensor.matmul(
                    ps,
                    lhsT=attnT_s[dl][:, 128 * st:128 * (st + 1)],
                    rhs=wo_s[dl][:, 512 * ec:512 * (ec + 1)],
                    start=(dl == 0), stop=(dl == 1))
            o_t = ob.tile([128, 512], BF16, tag="ob", name="ob")
            nc.vector.tensor_copy(o_t, ps)
            nc.sync.dma_start(
                out=out_d[128 * st:128 * (st + 1),
                          512 * ec:512 * (ec + 1)],
                in_=o_t)

        def emit_c_chunk(qh, jq):
            """Output projection for the 4 seq tiles of chunk jq_g=2qh+jq."""
            jq_g = 2 * qh + jq
            tags = [OT_TAGS[(0, jq)], OT_TAGS[(1, jq)]]
            i = 0
            for st in range(4 * jq_g, 4 * jq_g + 4):
                for ec in range(2):
                    emit_c_group(st, ec, tags[i % 2])
                    i += 1

        # ---------------- emission schedule ----------------
        # 1) K/Q head pair 0: all first-half groups (need only xt[0..3]),
        #    then all second-half groups.
        for e in (2, 0):
            for sc in range(4):
                emit_qk_halfA(e, sc)
        for e in (2, 0):
            for sc in range(4):
                emit_qk_halfB(e, sc)

        # 2) attention (hp=0, qh=0) with V st 0..7 and the head-pair-1
        #    K/Q groups needed early by section 4 interleaved
        ot00 = emit_ot_alloc(0)
        ins2 = {ki: [lambda st=ki: emit_v_group(st)] for ki in range(8)}
        ins2[4].append(lambda: emit_qk_single(3, 0))
        ins2[5].append(lambda: emit_qk_single(1, 0))
        ins2[6].append(lambda: emit_qk_single(1, 1))
        ins2[7].append(lambda: emit_qk_single(3, 1))
        emit_section(0, 0, 8, ot00, ins2)

        # 4) attention (hp=1, qh=0) with V st 8..13 interleaved and
        #    C(qh0, jq0) in the tail slots (tags free after slot-3/4
        #    normalizes)
        ot10 = emit_ot_alloc(0)
        ins4 = {ki: [lambda st=8 + ki: emit_v_group(st)] for ki in range(6)}
        cg0 = [(st, ec) for st in range(4) for ec in range(2)]
        ins4[5] = ins4[5] + [
            lambda st=st, ec=ec: emit_c_group(st, ec, OT_TAGS[((st + ec) % 2, 0)])
            for st, ec in cg0[0:2]]
        ins4[6] = [
            lambda st=st, ec=ec: emit_c_group(st, ec, OT_TAGS[((st + ec) % 2, 0)])
            for st, ec in cg0[2:5]]
        ins4[7] = [
            lambda st=st, ec=ec: emit_c_group(st, ec, OT_TAGS[((st + ec) % 2, 0)])
            for st, ec in cg0[5:8]]
        emit_section(1, 0, 8, ot10, ins4)

        # C(qh0, jq1) while its pb tags are free
        emit_c_chunk(0, 1)

        # 5) attention (hp=0, qh=1) with V st 14,15 and remaining K/Q
        #    pair-1 groups interleaved; fillers keep the PE dense
        ot01 = emit_ot_alloc(1)
        ins5 = {
            0: [lambda: emit_v_group(14)],
            1: [lambda: emit_v_group(15)],
            2: [lambda: emit_qk_single(1, 2)],
            3: [lambda: emit_qk_single(1, 3)],
            4: [lambda: emit_qk_single(3, 2)],
            5: [lambda: emit_qk_single(3, 3)],
        }
        for ki in range(6, 16):
            ins5[ki] = [lambda: emit_filler(ot01)]
        emit_section(0, 1, 16, ot01, ins5)

        # 6) attention (hp=1, qh=1); C(qh1, jq0) groups in the tail slots
        #    (their pb tags free after the slot-11/12 normalizes)
        ot11 = emit_ot_alloc(1)
        ins6 = {}
        for ki in range(1, 13):
            ins6[ki] = [lambda: emit_filler(ot11)]
        cg = [(st, ec) for st in range(8, 12) for ec in range(2)]
        for i, ki in enumerate((13, 14, 15)):
            part = cg[i * 3:(i + 1) * 3] if i < 2 else cg[6:]
            ins6[ki] = [
                lambda st=st, ec=ec: emit_c_group(
                    st, ec, OT_TAGS[(0, 0)] if (st + ec) % 2 == 0
                    else OT_TAGS[(1, 0)])
                for st, ec in part]
        emit_section(1, 1, 16, ot11, ins6)
        emit_c_chunk(1, 1)

    nc.compile()
    return nc


def _get_program():
    global _NC
    if _NC is None:
        _NC = _build_program()
    return _NC


def kernel(x, w_qkv, b_qkv, w_out, b_out):
    import ml_dtypes
    from concourse.bass_utils import run_bass_kernel_spmd

    BF = ml_dtypes.bfloat16
    x = np.asarray(x, dtype=np.float32)
    w_qkv = np.asarray(w_qkv, dtype=np.float32)
    b_qkv = np.asarray(b_qkv, dtype=np.float32)
    w_out = np.asarray(w_out, dtype=np.float32)
    b_out = np.asarray(b_out, dtype=np.float32)

    nc = _get_program()

    in_maps = []
    for c in range(N_CORES):
        b = c // 4
        g = c % 4
        hs = slice(g * EL, (g + 1) * EL)
        wq = w_qkv[0 * D:1 * D][hs]          # [256, 1024]
        wk = w_qkv[1 * D:2 * D][hs]
        wv = w_qkv[2 * D:3 * D][hs]
        bq = b_qkv[0 * D:1 * D][hs]
        bk = b_qkv[1 * D:2 * D][hs]
        bv = b_qkv[2 * D:3 * D][hs]
        bqk = np.concatenate([bq, bk])       # [512]
        wvx = np.zeros((D, HPC * 65), dtype=np.float32)
        bvx = np.zeros((1, HPC * 65), dtype=np.float32)
        for h in range(HPC):
            wvx[:, 65 * h:65 * h + 64] = wv[h * DH:(h + 1) * DH].T
            bvx[0, 65 * h:65 * h + 64] = bv[h * DH:(h + 1) * DH]
            bvx[0, 65 * h + 64] = 1.0
        in_maps.append({
            "xt": np.ascontiguousarray(x[b].T).astype(BF),             # [1024, 2048]
            "wqk": np.ascontiguousarray(np.concatenate([wq, wk]).T).astype(BF),
            "wv": wvx.astype(BF),                                      # [1024, 260]
            "bqk": np.ascontiguousarray(bqk.reshape(4, 128).T),        # [128, 4] f32
            "bv": bvx.astype(BF),                                      # [1, 260]
            "ones": np.ones((1, 512), dtype=BF),
            "wo": np.ascontiguousarray(w_out[:, hs].T).astype(BF),     # [256, 1024]
        })

    global _last_in_maps
    _last_in_maps = in_maps
    res = run_bass_kernel_spmd(nc, in_maps, list(range(N_CORES)))

    out = np.empty((B, S, D), dtype=np.float32)
    for b in range(B):
        acc = res.results[4 * b]["out"].astype(np.float32)
        for j in range(1, 4):
            acc = acc + res.results[4 * b + j]["out"].astype(np.float32)
        out[b] = acc + b_out[None, :]
    return out


# revision 40
# speedup vs baseline: 1.0418x; 1.0324x over previous
"""Causal self-attention (B=2, S=2048, D=1024, H=16, Dh=64) on 8 NeuronCores.

Sharding: core c -> batch b = c//4, head-group g = c%4 (heads 4g..4g+3).
Each core computes QKV projection for its 4 heads, causal attention
(scores kept transposed: [k, q] layout so no on-chip transposes are
needed), and a partial output projection over its local head dims.
Host sums the 4 partials per batch and adds b_out.

v7: bf16 operands (fp32 PSUM accumulation), software-pipelined attention
units — each unit's second-head attn@V is deferred one slot so the
in-order PE queue always has the next unit's scores to run while the
scalar engine computes exp — plus DMA striped across both HWDGE queues
in need order, split-contraction first projection groups, V/projection/
output-projection work interleaved into the attention windows, and the
output DMA spread across the kernel.
"""

import numpy as np
from contextlib import ExitStack

B = 2
S = 2048
D = 1024
NH = 16
DH = 64
N_CORES = 8
HPC = 4            # heads per core
EL = HPC * DH      # 256 local head dims per core
KD = D // 128      # 8 contraction chunks for projections
KT = S // 128      # 16 key tiles

_NC = None
_last_in_maps = None


def _build_program():
    import concourse.mybir as mybir
    import concourse.tile as tile
    from concourse import bacc

    F32 = mybir.dt.float32
    BF16 = mybir.dt.bfloat16
    Exp = mybir.ActivationFunctionType.Exp

    nc = bacc.Bacc("TRN2", target_bir_lowering=False, debug=False,
                   num_devices=N_CORES)

    xt_d = nc.dram_tensor("xt", [D, S], BF16, kind="ExternalInput")
    wqk_d = nc.dram_tensor("wqk", [D, 2 * EL], BF16, kind="ExternalInput")
    wv_d = nc.dram_tensor("wv", [D, HPC * 65], BF16, kind="ExternalInput")
    bqk_d = nc.dram_tensor("bqk", [128, 4], F32, kind="ExternalInput")
    ones_d = nc.dram_tensor("ones", [1, 512], BF16, kind="ExternalInput")
    bv_d = nc.dram_tensor("bv", [1, HPC * 65], BF16, kind="ExternalInput")
    wo_d = nc.dram_tensor("wo", [EL, D], BF16, kind="ExternalInput")
    out_d = nc.dram_tensor("out", [S, D], BF16, kind="ExternalOutput")

    with nc.allow_low_precision(reason="bf16 matmul operands, fp32 accum"), \
         tile.TileContext(nc) as tc, ExitStack() as ctx:
        const = ctx.enter_context(tc.tile_pool(name="const", bufs=1))
        work = ctx.enter_context(tc.tile_pool(name="work", bufs=1))
        pin = ctx.enter_context(tc.tile_pool(name="pin", bufs=1))
        psS = ctx.enter_context(tc.tile_pool(name="psS", bufs=1, space="PSUM"))
        pb = ctx.enter_context(tc.tile_pool(name="pb", bufs=1, space="PSUM"))
        pp = ctx.enter_context(tc.tile_pool(name="pp", bufs=4))
        sm = ctx.enter_context(tc.tile_pool(name="sm", bufs=2))
        tn = ctx.enter_context(tc.tile_pool(name="tn", bufs=3))
        hb = ctx.enter_context(tc.tile_pool(name="hb", bufs=1))
        ob = ctx.enter_context(tc.tile_pool(name="ob", bufs=4))

        # ---------------- input DMAs ----------------
        # Striped across both HWDGE queues in compute-need order: the
        # first-half projection groups need xt[0..3]+wqk[0..3] first.
        xt_s = [pin.tile([128, S], BF16, tag=f"xt{k}", name=f"xt{k}") for k in range(KD)]
        wqk_s = [pin.tile([128, 2 * EL], BF16, tag=f"wqk{k}", name=f"wqk{k}") for k in range(KD)]
        wv_s = [pin.tile([128, HPC * 65], BF16, tag=f"wv{k}", name=f"wv{k}") for k in range(KD)]
        for k in range(0, KD, 2):
            nc.sync.dma_start(out=xt_s[k], in_=xt_d[128 * k:128 * (k + 1), :])
        for k in range(4):
            nc.scalar.dma_start(out=wqk_s[k], in_=wqk_d[128 * k:128 * (k + 1), :])
        for k in range(1, 4, 2):
            nc.scalar.dma_start(out=xt_s[k], in_=xt_d[128 * k:128 * (k + 1), :])
        for k in range(4, KD):
            nc.scalar.dma_start(out=wqk_s[k], in_=wqk_d[128 * k:128 * (k + 1), :])
        for k in range(5, KD, 2):
            nc.scalar.dma_start(out=xt_s[k], in_=xt_d[128 * k:128 * (k + 1), :])
        for k in range(KD):
            nc.scalar.dma_start(out=wv_s[k], in_=wv_d[128 * k:128 * (k + 1), :])
        wo_s = [const.tile([128, D], BF16, tag=f"wo{i}", name=f"wo{i}") for i in range(2)]
        for i in range(2):
            nc.scalar.dma_start(out=wo_s[i], in_=wo_d[128 * i:128 * (i + 1), :])
        bqk_s = const.tile([128, 4], F32, tag="bqk", name="bqk")
        nc.sync.dma_start(out=bqk_s, in_=bqk_d[:, :])
        bv_s = const.tile([1, HPC * 65], BF16, tag="bv", name="bv")
        nc.sync.dma_start(out=bv_s, in_=bv_d[:, :])
        ones_s = const.tile([1, 512], BF16, tag="ones", name="ones")
        nc.sync.dma_start(out=ones_s, in_=ones_d[:, :])
        # ones living on partition 64, for the denominator broadcast matmul
        ones64_s = const.tile([65, 64], BF16, tag="ones64", name="ones64")
        nc.sync.dma_start(out=ones64_s[64:65, :], in_=ones_d[0:1, 0:64])
        # zero weights for harmless PE keep-warm filler matmuls
        zeros_s = const.tile([128, 65], BF16, tag="zeros", name="zeros")
        nc.gpsimd.memset(zeros_s[:, :], 0.0)

        # ---------------- persistent SBUF tensors ----------------
        # qkT: e-tiles 0,1 = Q (head pairs 0,1), 2,3 = K
        qk_s = [work.tile([128, S], BF16, tag=f"qk{e}", name=f"qk{e}") for e in range(4)]
        # V augmented: per key-tile [128, 4*65]; col 64 of each head = 1.0
        vaug_s = [work.tile([128, HPC * 65], BF16, tag=f"va{t}", name=f"va{t}") for t in range(KT)]
        # normalized attn output, transposed: [d_local, s]
        attnT_s = [work.tile([128, S], BF16, tag=f"at{d}", name=f"at{d}") for d in range(2)]

        # ---------------- emission helpers ----------------
        nps = [0]

        def stag():
            nps[0] += 1
            return f"s{nps[0] % 2}"

        h_sb = {}

        def emit_qk_halfA(e, sc):
            """First half of the contraction (k 0..3) + bias, parked in SBUF."""
            ps = psS.tile([128, 512], F32, tag=stag(), name="psA")
            for k in range(4):
                nc.tensor.matmul(
                    ps,
                    lhsT=wqk_s[k][:, 128 * e:128 * (e + 1)],
                    rhs=xt_s[k][:, 512 * sc:512 * (sc + 1)],
                    start=(k == 0), stop=(k == 3))
            h = hb.tile([128, 512], F32, tag=f"h{e}{sc}", name=f"h{e}{sc}")
            nc.vector.tensor_scalar_add(out=h, in0=ps,
                                        scalar1=bqk_s[:, e:e + 1])
            h_sb[(e, sc)] = h

        def emit_qk_halfB(e, sc):
            """Second half (k 4..7); combined with the parked first half."""
            ps = psS.tile([128, 512], F32, tag=stag(), name="psB")
            for k in range(4, KD):
                nc.tensor.matmul(
                    ps,
                    lhsT=wqk_s[k][:, 128 * e:128 * (e + 1)],
                    rhs=xt_s[k][:, 512 * sc:512 * (sc + 1)],
                    start=(k == 4), stop=(k == KD - 1))
            nc.vector.tensor_add(
                out=qk_s[e][:, 512 * sc:512 * (sc + 1)],
                in0=ps, in1=h_sb[(e, sc)])

        def emit_qk_single(e, sc):
            ps = psS.tile([128, 512], F32, tag=stag(), name="psQ")
            for k in range(KD):
                nc.tensor.matmul(
                    ps,
                    lhsT=wqk_s[k][:, 128 * e:128 * (e + 1)],
                    rhs=xt_s[k][:, 512 * sc:512 * (sc + 1)],
                    start=(k == 0), stop=(k == KD - 1))
            nc.vector.tensor_scalar_add(
                out=qk_s[e][:, 512 * sc:512 * (sc + 1)], in0=ps,
                scalar1=bqk_s[:, e:e + 1])

        def emit_v_group(st):
            ps = psS.tile([128, HPC * 65], F32, tag=stag(), name="psV")
            for k in range(KD):
                nc.tensor.matmul(
                    ps,
                    lhsT=xt_s[k][:, 128 * st:128 * (st + 1)],
                    rhs=wv_s[k],
                    start=(k == 0), stop=False)
            nc.tensor.matmul(ps, lhsT=ones_s[0:1, 0:128], rhs=bv_s,
                             start=False, stop=True)
            nc.vector.tensor_copy(vaug_s[st], ps)

        # attention state per (hp, qh): dict (h2, jq) -> psum tile
        OT_TAGS = {(0, 0): "o00", (1, 0): "o10", (0, 1): "o01", (1, 1): "o11"}

        def emit_ot_alloc(qh):
            return {
                (h2, jq): pb.tile([65, 512], F32, tag=OT_TAGS[(h2, jq)],
                                  name=f"ot{h2}{jq}")
                for h2 in range(2) for jq in range(2)
            }

        def emit_normalize_cols(hp, h2, t, jq_g, c0, c1):
            """Normalize ot columns [c0:c1) into attnT."""
            w = c1 - c0
            den = sm.tile([65, 512], BF16, tag="den", name="den")
            nc.vector.tensor_copy(den[64:65, 0:w], t[64:65, c0:c1])
            rb_ps = psS.tile([64, 512], F32, tag=stag(), name="rb")
            nc.tensor.matmul(rb_ps[:, 0:w], lhsT=ones64_s[64:65, :],
                             rhs=den[64:65, 0:w], start=True, stop=True)
            rb_sb = sm.tile([64, 512], F32, tag="rbs", name="rbs")
            nc.vector.reciprocal_approx_fast(out=rb_sb[:, 0:w],
                                             in_=rb_ps[:, 0:w])
            a0 = 512 * jq_g + c0
            if h2 == 0:
                nc.vector.tensor_mul(
                    out=attnT_s[hp][0:64, a0:a0 + w],
                    in0=t[0:64, c0:c1], in1=rb_sb[:, 0:w])
            else:
                t_n = tn.tile([64, 512], BF16, tag="tn", name="tn")
                nc.vector.tensor_mul(out=t_n[:, 0:w], in0=t[0:64, c0:c1],
                                     in1=rb_sb[:, 0:w])
                nc.sync.dma_start(
                    out=attnT_s[hp][64:128, a0:a0 + w],
                    in_=t_n[:, 0:w])

        def emit_normalize(hp, qh, h2, jq, ot):
            emit_normalize_cols(hp, h2, ot[(h2, jq)], 2 * qh + jq, 0, 512)

        def emit_scores_pair(hp, qh, ki):
            """Scores for both heads of the pair, interleaved h0/h1 per
            q-chunk so the 64-row matmuls overlap on row groups 0/64."""
            qt = qk_s[hp]
            kt_ = qk_s[2 + hp]
            s_ps = [psS.tile([128, 1024], F32, tag=f"s{h2}", name=f"s{h2}")
                    for h2 in range(2)]
            for qq in range(2):
                q0 = 1024 * qh + 512 * qq
                if q0 + 512 <= 128 * ki:
                    continue
                for h2 in range(2):
                    pbase = 64 * h2
                    nc.tensor.matmul(
                        s_ps[h2][:, 512 * qq:512 * (qq + 1)],
                        lhsT=kt_[pbase:pbase + 64, 128 * ki:128 * (ki + 1)],
                        rhs=qt[pbase:pbase + 64, q0:q0 + 512],
                        start=True, stop=True)
            return s_ps

        def emit_exp(qh, ki, s_ps):
            off = max(0, 128 * ki - 1024 * qh)
            p_t = pp.tile([128, 1024], BF16, tag="p", name="pt")
            nc.scalar.activation(
                out=p_t[:, off:1024], in_=s_ps[:, off:1024],
                func=Exp, scale=0.125)
            if 128 * ki >= 1024 * qh:
                dof = 128 * ki - 1024 * qh
                nc.gpsimd.affine_select(
                    out=p_t[:, dof:dof + 128],
                    in_=p_t[:, dof:dof + 128],
                    compare_op=mybir.AluOpType.is_ge, fill=0.0,
                    base=0, pattern=[[1, 128]], channel_multiplier=-1)
            return p_t

        def emit_attnv(hp, qh, ki, h2, p_t, ot):
            h = 2 * hp + h2
            for jq in range(2):
                jq_g = 2 * qh + jq
                if ki > 4 * jq_g + 3:
                    continue
                q0g = max(512 * jq_g, 128 * ki)
                nc.tensor.matmul(
                    ot[(h2, jq)][:, q0g - 512 * jq_g:512],
                    lhsT=vaug_s[ki][:, 65 * h:65 * h + 65],
                    rhs=p_t[:, q0g - 1024 * qh:
                                512 * (jq_g + 1) - 1024 * qh],
                    start=(ki == 0), stop=(ki == 4 * jq_g + 3),
                    skip_group_check=True)
            for jq in range(2):
                if ki == 4 * (2 * qh + jq) + 3:
                    emit_normalize(hp, qh, h2, jq, ot)

        def emit_section(hp, qh, nki, ot, inserts):
            """Software-pipelined attention section: the h2=1 attn@V of
            each unit is deferred one slot so the PE has the next unit's
            scores available while ACT runs exp."""
            pend = None
            for ki in range(nki):
                ins = inserts.get(ki)
                if ins:
                    for f in ins:
                        f()
                s01 = emit_scores_pair(hp, qh, ki)
                if pend is not None:
                    emit_attnv(hp, qh, pend[0], 1, pend[1], ot)
                p0 = emit_exp(qh, ki, s01[0])
                emit_attnv(hp, qh, ki, 0, p0, ot)
                p1 = emit_exp(qh, ki, s01[1])
                pend = (ki, p1)
            emit_attnv(hp, qh, pend[0], 1, pend[1], ot)

        def emit_filler(ot):
            """Zero-weight accumulates into the live jq=1 groups: keep the
            PE issue stream dense (HAM stays warm) without changing math."""
            for h2 in range(2):
                nc.tensor.matmul(
                    ot[(h2, 1)][:, 0:512],
                    lhsT=zeros_s[:, 0:65],
                    rhs=qk_s[h2][:, 0:512],
                    start=False, stop=False,
                    skip_group_check=True)

        def emit_c_group(st, ec, tag):
            ps = pb.tile([128, 512], F32, tag=tag, name="psC")
            for dl in range(2):
                nc.tensor.matmul(
                    ps,
                    lhsT=attnT_s[dl][:, 128 * st:128 * (st + 1)],
                    rhs=wo_s[dl][:, 512 * ec:512 * (ec + 1)],
                    start=(dl == 0), stop=(dl == 1))
            o_t = ob.tile([128, 512], BF16, tag="ob", name="ob")
            nc.vector.tensor_copy(o_t, ps)
            nc.sync.dma_start(
                out=out_d[128 * st:128 * (st + 1),
                          512 * ec:512 * (ec + 1)],
                in_=o_t)

        def emit_c_chunk(qh, jq):
            """Output projection for the 4 seq tiles of chunk jq_g=2qh+jq."""
            jq_g = 2 * qh + jq
            tags = [OT_TAGS[(0, jq)], OT_TAGS[(1, jq)]]
            i = 0
            for st in range(4 * jq_g, 4 * jq_g + 4):
                for ec in range(2):
                    emit_c_group(st, ec, tags[i % 2])
                    i += 1

        # ---------------- emission schedule ----------------
        # 1) K/Q head pair 0: all first-half groups (need only xt[0..3]),
        #    then all second-half groups.
        for e in (2, 0):
            for sc in range(4):
                emit_qk_halfA(e, sc)
        # only the groups attention needs first; sc 2,3 (used by the
        # qh=1 sections only) are deferred into section 4's slots
        for e, sc in ((2, 0), (2, 1), (0, 0), (0, 1)):
            emit_qk_halfB(e, sc)

        # 2) attention (hp=0, qh=0) with V st 0..7 and the head-pair-1
        #    K/Q groups needed early by section 4 interleaved
        ot00 = emit_ot_alloc(0)
        ins2 = {ki: [lambda st=ki: emit_v_group(st)] for ki in range(8)}
        ins2[4].append(lambda: emit_qk_single(3, 0))
        ins2[5].append(lambda: emit_qk_single(1, 0))
        ins2[6].append(lambda: emit_qk_single(1, 1))
        ins2[7].append(lambda: emit_qk_single(3, 1))
        emit_section(0, 0, 8, ot00, ins2)

        # 4) attention (hp=1, qh=0) with V st 8..13 interleaved and
        #    C(qh0, jq0) in the tail slots (tags free after slot-3/4
        #    normalizes)
        ot10 = emit_ot_alloc(0)
        ins4 = {ki: [lambda st=8 + ki: emit_v_group(st)] for ki in range(6)}
        for ki, (e, sc) in enumerate(((0, 2), (0, 3), (2, 2), (2, 3))):
            ins4[ki].append(lambda e=e, sc=sc: emit_qk_halfB(e, sc))
        cg0 = [(st, ec) for st in range(4) for ec in range(2)]
        ins4[5] = ins4[5] + [
            lambda st=st, ec=ec: emit_c_group(st, ec, OT_TAGS[((st + ec) % 2, 0)])
            for st, ec in cg0[0:2]]
        ins4[6] = [
            lambda st=st, ec=ec: emit_c_group(st, ec, OT_TAGS[((st + ec) % 2, 0)])
            for st, ec in cg0[2:5]]
        ins4[7] = [
            lambda st=st, ec=ec: emit_c_group(st, ec, OT_TAGS[((st + ec) % 2, 0)])
            for st, ec in cg0[5:8]]
        emit_section(1, 0, 8, ot10, ins4)

        # C(qh0, jq1) while its pb tags are free
        emit_c_chunk(0, 1)

        # 5) attention (hp=0, qh=1) with V st 14,15 and remaining K/Q
        #    pair-1 groups interleaved; fillers keep the PE dense
        ot01 = emit_ot_alloc(1)
        ins5 = {
            0: [lambda: emit_v_group(14)],
            1: [lambda: emit_v_group(15)],
            2: [lambda: emit_qk_single(1, 2)],
            3: [lambda: emit_qk_single(1, 3)],
            4: [lambda: emit_qk_single(3, 2)],
            5: [lambda: emit_qk_single(3, 3)],
        }
        for ki in range(6, 16):
            ins5[ki] = [lambda: emit_filler(ot01)]
        emit_section(0, 1, 16, ot01, ins5)

        # 6) attention (hp=1, qh=1); C(qh1, jq0) groups in the tail slots
        #    (their pb tags free after the slot-11/12 normalizes)
        ot11 = emit_ot_alloc(1)
        ins6 = {}
        for ki in range(1, 13):
            ins6[ki] = [lambda: emit_filler(ot11)]
        cg = [(st, ec) for st in range(8, 12) for ec in range(2)]
        for i, ki in enumerate((13, 14, 15)):
            part = cg[i * 3:(i + 1) * 3] if i < 2 else cg[6:]
            ins6[ki] = [
                lambda st=st, ec=ec: emit_c_group(
                    st, ec, OT_TAGS[(0, 0)] if (st + ec) % 2 == 0
                    else OT_TAGS[(1, 0)])
                for st, ec in part]
        emit_section(1, 1, 16, ot11, ins6)
        emit_c_chunk(1, 1)

    nc.compile()
    return nc


def _get_program():
    global _NC
    if _NC is None:
        _NC = _build_program()
    return _NC


def kernel(x, w_qkv, b_qkv, w_out, b_out):
    import ml_dtypes
    from concourse.bass_utils import run_bass_kernel_spmd

    BF = ml_dtypes.bfloat16
    x = np.asarray(x, dtype=np.float32)
    w_qkv = np.asarray(w_qkv, dtype=np.float32)
    b_qkv = np.asarray(b_qkv, dtype=np.float32)
    w_out = np.asarray(w_out, dtype=np.float32)
    b_out = np.asarray(b_out, dtype=np.float32)

    nc = _get_program()

    in_maps = []
    for c in range(N_CORES):
        b = c // 4
        g = c % 4
        hs = slice(g * EL, (g + 1) * EL)
        wq = w_qkv[0 * D:1 * D][hs]          # [256, 1024]
        wk = w_qkv[1 * D:2 * D][hs]
        wv = w_qkv[2 * D:3 * D][hs]
        bq = b_qkv[0 * D:1 * D][hs]
        bk = b_qkv[1 * D:2 * D][hs]
        bv = b_qkv[2 * D:3 * D][hs]
        bqk = np.concatenate([bq, bk])       # [512]
        wvx = np.zeros((D, HPC * 65), dtype=np.float32)
        bvx = np.zeros((1, HPC * 65), dtype=np.float32)
        for h in range(HPC):
            wvx[:, 65 * h:65 * h + 64] = wv[h * DH:(h + 1) * DH].T
            bvx[0, 65 * h:65 * h + 64] = bv[h * DH:(h + 1) * DH]
            bvx[0, 65 * h + 64] = 1.0
        in_maps.append({
            "xt": np.ascontiguousarray(x[b].T).astype(BF),             # [1024, 2048]
            "wqk": np.ascontiguousarray(np.concatenate([wq, wk]).T).astype(BF),
            "wv": wvx.astype(BF),                                      # [1024, 260]
            "bqk": np.ascontiguousarray(bqk.reshape(4, 128).T),        # [128, 4] f32
            "bv": bvx.astype(BF),                                      # [1, 260]
            "ones": np.ones((1, 512), dtype=BF),
            "wo": np.ascontiguousarray(w_out[:, hs].T).astype(BF),     # [256, 1024]
        })

    global _last_in_maps
    _last_in_maps = in_maps
    res = run_bass_kernel_spmd(nc, in_maps, list(range(N_CORES)))

    out = np.empty((B, S, D), dtype=np.float32)
    for b in range(B):
        acc = res.results[4 * b]["out"].astype(np.float32)
        for j in range(1, 4):
            acc = acc + res.results[4 * b + j]["out"].astype(np.float32)
        out[b] = acc + b_out[None, :]
    return out


# revision 46
# speedup vs baseline: 1.0618x; 1.0193x over previous
"""Causal self-attention (B=2, S=2048, D=1024, H=16, Dh=64) on 8 NeuronCores.

Sharding: core c -> batch b = c//4, head-group g = c%4 (heads 4g..4g+3).
Each core computes QKV projection for its 4 heads, causal attention
(scores kept transposed: [k, q] layout so no on-chip transposes are
needed), and a partial output projection over its local head dims.
Host sums the 4 partials per batch and adds b_out.

v7: bf16 operands (fp32 PSUM accumulation), software-pipelined attention
units — each unit's second-head attn@V is deferred one slot so the
in-order PE queue always has the next unit's scores to run while the
scalar engine computes exp — plus DMA striped across both HWDGE queues
in need order, split-contraction first projection groups, V/projection/
output-projection work interleaved into the attention windows, and the
output DMA spread across the kernel.
"""

import numpy as np
from contextlib import ExitStack

B = 2
S = 2048
D = 1024
NH = 16
DH = 64
N_CORES = 8
HPC = 4            # heads per core
EL = HPC * DH      # 256 local head dims per core
KD = D // 128      # 8 contraction chunks for projections
KT = S // 128      # 16 key tiles

_NC = None
_last_in_maps = None


def _build_program():
    import concourse.mybir as mybir
    import concourse.tile as tile
    from concourse import bacc

    F32 = mybir.dt.float32
    BF16 = mybir.dt.bfloat16
    Exp = mybir.ActivationFunctionType.Exp

    nc = bacc.Bacc("TRN2", target_bir_lowering=False, debug=False,
                   num_devices=N_CORES)

    xt_d = nc.dram_tensor("xt", [D, S], BF16, kind="ExternalInput")
    wqk_d = nc.dram_tensor("wqk", [D, 2 * EL], BF16, kind="ExternalInput")
    wv_d = nc.dram_tensor("wv", [D, HPC * 65], BF16, kind="ExternalInput")
    bqk_d = nc.dram_tensor("bqk", [128, 4], F32, kind="ExternalInput")
    ones_d = nc.dram_tensor("ones", [1, 512], BF16, kind="ExternalInput")
    bv_d = nc.dram_tensor("bv", [1, HPC * 65], BF16, kind="ExternalInput")
    wo_d = nc.dram_tensor("wo", [EL, D], BF16, kind="ExternalInput")
    out_d = nc.dram_tensor("out", [S, D], BF16, kind="ExternalOutput")

    with nc.allow_low_precision(reason="bf16 matmul operands, fp32 accum"), \
         tile.TileContext(nc) as tc, ExitStack() as ctx:
        const = ctx.enter_context(tc.tile_pool(name="const", bufs=1))
        work = ctx.enter_context(tc.tile_pool(name="work", bufs=1))
        pin = ctx.enter_context(tc.tile_pool(name="pin", bufs=1))
        psS = ctx.enter_context(tc.tile_pool(name="psS", bufs=1, space="PSUM"))
        pb = ctx.enter_context(tc.tile_pool(name="pb", bufs=1, space="PSUM"))
        pp = ctx.enter_context(tc.tile_pool(name="pp", bufs=4))
        sm = ctx.enter_context(tc.tile_pool(name="sm", bufs=2))
        tn = ctx.enter_context(tc.tile_pool(name="tn", bufs=3))
        hb = ctx.enter_context(tc.tile_pool(name="hb", bufs=1))
        ob = ctx.enter_context(tc.tile_pool(name="ob", bufs=4))

        # ---------------- input DMAs ----------------
        # Striped across both HWDGE queues in compute-need order: the
        # first-half projection groups need xt[0..3]+wqk[0..3] first.
        xt_s = [pin.tile([128, S], BF16, tag=f"xt{k}", name=f"xt{k}") for k in range(KD)]
        wqk_s = [pin.tile([128, 2 * EL], BF16, tag=f"wqk{k}", name=f"wqk{k}") for k in range(KD)]
        wv_s = [pin.tile([128, HPC * 65], BF16, tag=f"wv{k}", name=f"wv{k}") for k in range(KD)]
        for k in range(0, KD, 2):
            nc.sync.dma_start(out=xt_s[k], in_=xt_d[128 * k:128 * (k + 1), :])
        for k in range(4):
            nc.scalar.dma_start(out=wqk_s[k], in_=wqk_d[128 * k:128 * (k + 1), :])
        for k in range(1, 4, 2):
            nc.scalar.dma_start(out=xt_s[k], in_=xt_d[128 * k:128 * (k + 1), :])
        for k in range(4, KD):
            nc.scalar.dma_start(out=wqk_s[k], in_=wqk_d[128 * k:128 * (k + 1), :])
        for k in range(5, KD, 2):
            nc.scalar.dma_start(out=xt_s[k], in_=xt_d[128 * k:128 * (k + 1), :])
        for k in range(KD):
            nc.scalar.dma_start(out=wv_s[k], in_=wv_d[128 * k:128 * (k + 1), :])
        wo_s = [const.tile([128, D], BF16, tag=f"wo{i}", name=f"wo{i}") for i in range(2)]
        for i in range(2):
            nc.scalar.dma_start(out=wo_s[i], in_=wo_d[128 * i:128 * (i + 1), :])
        bqk_s = const.tile([128, 4], F32, tag="bqk", name="bqk")
        nc.sync.dma_start(out=bqk_s, in_=bqk_d[:, :])
        bv_s = const.tile([1, HPC * 65], BF16, tag="bv", name="bv")
        nc.sync.dma_start(out=bv_s, in_=bv_d[:, :])
        ones_s = const.tile([1, 512], BF16, tag="ones", name="ones")
        nc.sync.dma_start(out=ones_s, in_=ones_d[:, :])
        # ones living on partition 64, for the denominator broadcast matmul
        ones64_s = const.tile([65, 64], BF16, tag="ones64", name="ones64")
        nc.sync.dma_start(out=ones64_s[64:65, :], in_=ones_d[0:1, 0:64])
        # zero weights for harmless PE keep-warm filler matmuls
        zeros_s = const.tile([128, 65], BF16, tag="zeros", name="zeros")
        nc.gpsimd.memset(zeros_s[:, :], 0.0)

        # ---------------- persistent SBUF tensors ----------------
        # qkT: e-tiles 0,1 = Q (head pairs 0,1), 2,3 = K
        qk_s = [work.tile([128, S], BF16, tag=f"qk{e}", name=f"qk{e}") for e in range(4)]
        # V augmented: per key-tile [128, 4*65]; col 64 of each head = 1.0
        vaug_s = [work.tile([128, HPC * 65], BF16, tag=f"va{t}", name=f"va{t}") for t in range(KT)]
        # normalized attn output, transposed: [d_local, s]
        attnT_s = [work.tile([128, S], BF16, tag=f"at{d}", name=f"at{d}") for d in range(2)]

        # ---------------- emission helpers ----------------
        nps = [0]

        def stag():
            nps[0] += 1
            return f"s{nps[0] % 2}"

        h_sb = {}

        def emit_qk_halfA(e, sc):
            """First half of the contraction (k 0..3) + bias, parked in SBUF."""
            ps = psS.tile([128, 512], F32, tag=stag(), name="psA")
            for k in range(4):
                nc.tensor.matmul(
                    ps,
                    lhsT=wqk_s[k][:, 128 * e:128 * (e + 1)],
                    rhs=xt_s[k][:, 512 * sc:512 * (sc + 1)],
                    start=(k == 0), stop=(k == 3))
            h = hb.tile([128, 512], F32, tag=f"h{e}{sc}", name=f"h{e}{sc}")
            nc.vector.tensor_scalar_add(out=h, in0=ps,
                                        scalar1=bqk_s[:, e:e + 1])
            h_sb[(e, sc)] = h

        def emit_qk_halfB(e, sc):
            """Second half (k 4..7); combined with the parked first half."""
            ps = psS.tile([128, 512], F32, tag=stag(), name="psB")
            for k in range(4, KD):
                nc.tensor.matmul(
                    ps,
                    lhsT=wqk_s[k][:, 128 * e:128 * (e + 1)],
                    rhs=xt_s[k][:, 512 * sc:512 * (sc + 1)],
                    start=(k == 4), stop=(k == KD - 1))
            nc.vector.tensor_add(
                out=qk_s[e][:, 512 * sc:512 * (sc + 1)],
                in0=ps, in1=h_sb[(e, sc)])

        def emit_qk_single(e, sc):
            ps = psS.tile([128, 512], F32, tag=stag(), name="psQ")
            for k in range(KD):
                nc.tensor.matmul(
                    ps,
                    lhsT=wqk_s[k][:, 128 * e:128 * (e + 1)],
                    rhs=xt_s[k][:, 512 * sc:512 * (sc + 1)],
                    start=(k == 0), stop=(k == KD - 1))
            nc.vector.tensor_scalar_add(
                out=qk_s[e][:, 512 * sc:512 * (sc + 1)], in0=ps,
                scalar1=bqk_s[:, e:e + 1])

        def emit_v_group(st):
            ps = psS.tile([128, HPC * 65], F32, tag=stag(), name="psV")
            for k in range(KD):
                nc.tensor.matmul(
                    ps,
                    lhsT=xt_s[k][:, 128 * st:128 * (st + 1)],
                    rhs=wv_s[k],
                    start=(k == 0), stop=False)
            nc.tensor.matmul(ps, lhsT=ones_s[0:1, 0:128], rhs=bv_s,
                             start=False, stop=True)
            nc.vector.tensor_copy(vaug_s[st], ps)

        # attention state per (hp, qh): dict (h2, jq) -> psum tile
        OT_TAGS = {(0, 0): "o00", (1, 0): "o10", (0, 1): "o01", (1, 1): "o11"}

        def emit_ot_alloc(qh):
            return {
                (h2, jq): pb.tile([65, 512], F32, tag=OT_TAGS[(h2, jq)],
                                  name=f"ot{h2}{jq}")
                for h2 in range(2) for jq in range(2)
            }

        def emit_normalize_cols(hp, h2, t, jq_g, c0, c1):
            """Normalize ot columns [c0:c1) into attnT."""
            w = c1 - c0
            den = sm.tile([65, 512], BF16, tag="den", name="den")
            nc.vector.tensor_copy(den[64:65, 0:w], t[64:65, c0:c1])
            rb_ps = psS.tile([64, 512], F32, tag=stag(), name="rb")
            nc.tensor.matmul(rb_ps[:, 0:w], lhsT=ones64_s[64:65, :],
                             rhs=den[64:65, 0:w], start=True, stop=True)
            rb_sb = sm.tile([64, 512], F32, tag="rbs", name="rbs")
            nc.vector.reciprocal_approx_fast(out=rb_sb[:, 0:w],
                                             in_=rb_ps[:, 0:w])
            a0 = 512 * jq_g + c0
            if h2 == 0:
                nc.vector.tensor_mul(
                    out=attnT_s[hp][0:64, a0:a0 + w],
                    in0=t[0:64, c0:c1], in1=rb_sb[:, 0:w])
            else:
                t_n = tn.tile([64, 512], BF16, tag="tn", name="tn")
                nc.vector.tensor_mul(out=t_n[:, 0:w], in0=t[0:64, c0:c1],
                                     in1=rb_sb[:, 0:w])
                nc.sync.dma_start(
                    out=attnT_s[hp][64:128, a0:a0 + w],
                    in_=t_n[:, 0:w])

        def emit_normalize(hp, qh, h2, jq, ot):
            emit_normalize_cols(hp, h2, ot[(h2, jq)], 2 * qh + jq, 0, 512)

        def emit_scores_pair(hp, qh, ki):
            """Scores for both heads of the pair, interleaved h0/h1 per
            q-chunk so the 64-row matmuls overlap on row groups 0/64."""
            qt = qk_s[hp]
            kt_ = qk_s[2 + hp]
            s_ps = [psS.tile([128, 1024], F32, tag=f"s{h2}", name=f"s{h2}")
                    for h2 in range(2)]
            for qq in range(2):
                q0 = 1024 * qh + 512 * qq
                if q0 + 512 <= 128 * ki:
                    continue
                for h2 in range(2):
                    pbase = 64 * h2
                    nc.tensor.matmul(
                        s_ps[h2][:, 512 * qq:512 * (qq + 1)],
                        lhsT=kt_[pbase:pbase + 64, 128 * ki:128 * (ki + 1)],
                        rhs=qt[pbase:pbase + 64, q0:q0 + 512],
                        start=True, stop=True)
            return s_ps

        def emit_exp(qh, ki, s_ps):
            off = max(0, 128 * ki - 1024 * qh)
            p_t = pp.tile([128, 1024], BF16, tag="p", name="pt")
            nc.scalar.activation(
                out=p_t[:, off:1024], in_=s_ps[:, off:1024],
                func=Exp, scale=0.125)
            if 128 * ki >= 1024 * qh:
                dof = 128 * ki - 1024 * qh
                nc.gpsimd.affine_select(
                    out=p_t[:, dof:dof + 128],
                    in_=p_t[:, dof:dof + 128],
                    compare_op=mybir.AluOpType.is_ge, fill=0.0,
                    base=0, pattern=[[1, 128]], channel_multiplier=-1)
            return p_t

        def emit_attnv(hp, qh, ki, h2, p_t, ot):
            h = 2 * hp + h2
            for jq in range(2):
                jq_g = 2 * qh + jq
                if ki > 4 * jq_g + 3:
                    continue
                q0g = max(512 * jq_g, 128 * ki)
                nc.tensor.matmul(
                    ot[(h2, jq)][:, q0g - 512 * jq_g:512],
                    lhsT=vaug_s[ki][:, 65 * h:65 * h + 65],
                    rhs=p_t[:, q0g - 1024 * qh:
                                512 * (jq_g + 1) - 1024 * qh],
                    start=(ki == 0), stop=(ki == 4 * jq_g + 3),
                    skip_group_check=True)
            for jq in range(2):
                if ki == 4 * (2 * qh + jq) + 3:
                    emit_normalize(hp, qh, h2, jq, ot)

        def emit_section(hp, qh, nki, ot, inserts, carry_in=None):
            """Software-pipelined attention section: the h2=1 attn@V of
            each unit is deferred one slot so the PE has the next unit's
            scores available while ACT runs exp. The final deferred unit
            is returned as a closure so the NEXT section can run it after
            its own first scores (cross-section pipelining)."""
            pend = None
            for ki in range(nki):
                ins = inserts.get(ki)
                if ins:
                    for f in ins:
                        f()
                s01 = emit_scores_pair(hp, qh, ki)
                if ki == 0 and carry_in is not None:
                    carry_in()
                if pend is not None:
                    emit_attnv(hp, qh, pend[0], 1, pend[1], ot)
                p0 = emit_exp(qh, ki, s01[0])
                emit_attnv(hp, qh, ki, 0, p0, ot)
                p1 = emit_exp(qh, ki, s01[1])
                pend = (ki, p1)
            last = pend
            return lambda: emit_attnv(hp, qh, last[0], 1, last[1], ot)

        def emit_filler(ot):
            """Zero-weight accumulates into the live jq=1 groups: keep the
            PE issue stream dense (HAM stays warm) without changing math."""
            for h2 in range(2):
                nc.tensor.matmul(
                    ot[(h2, 1)][:, 0:512],
                    lhsT=zeros_s[:, 0:65],
                    rhs=qk_s[h2][:, 0:512],
                    start=False, stop=False,
                    skip_group_check=True)

        def emit_c_group(st, ec, tag):
            ps = pb.tile([128, 512], F32, tag=tag, name="psC")
            for dl in range(2):
                nc.tensor.matmul(
                    ps,
                    lhsT=attnT_s[dl][:, 128 * st:128 * (st + 1)],
                    rhs=wo_s[dl][:, 512 * ec:512 * (ec + 1)],
                    start=(dl == 0), stop=(dl == 1))
            o_t = ob.tile([128, 512], BF16, tag="ob", name="ob")
            nc.vector.tensor_copy(o_t, ps)
            nc.sync.dma_start(
                out=out_d[128 * st:128 * (st + 1),
                          512 * ec:512 * (ec + 1)],
                in_=o_t)

        def emit_c_chunk(qh, jq):
            """Output projection for the 4 seq tiles of chunk jq_g=2qh+jq."""
            jq_g = 2 * qh + jq
            tags = [OT_TAGS[(0, jq)], OT_TAGS[(1, jq)]]
            i = 0
            for st in range(4 * jq_g, 4 * jq_g + 4):
                for ec in range(2):
                    emit_c_group(st, ec, tags[i % 2])
                    i += 1

        # ---------------- emission schedule ----------------
        # 1) K/Q head pair 0: all first-half groups (need only xt[0..3]),
        #    then all second-half groups.
        for e in (2, 0):
            for sc in range(4):
                emit_qk_halfA(e, sc)
        # only the groups attention needs first; sc 2,3 (used by the
        # qh=1 sections only) are deferred into section 4's slots
        for e, sc in ((2, 0), (2, 1), (0, 0), (0, 1)):
            emit_qk_halfB(e, sc)

        # 2) attention (hp=0, qh=0) with V st 0..7 and the head-pair-1
        #    K/Q groups needed early by section 4 interleaved
        ot00 = emit_ot_alloc(0)
        ins2 = {ki: [lambda st=ki: emit_v_group(st)] for ki in range(8)}
        ins2[4].append(lambda: emit_qk_single(3, 0))
        ins2[5].append(lambda: emit_qk_single(1, 0))
        ins2[6].append(lambda: emit_qk_single(1, 1))
        ins2[7].append(lambda: emit_qk_single(3, 1))
        c2 = emit_section(0, 0, 8, ot00, ins2)

        # 4) attention (hp=1, qh=0) with V st 8..13 interleaved and
        #    C(qh0, jq0) in the tail slots (tags free after slot-3/4
        #    normalizes)
        ot10 = emit_ot_alloc(0)
        ins4 = {ki: [lambda st=8 + ki: emit_v_group(st)] for ki in range(6)}
        for ki, (e, sc) in enumerate(((0, 2), (0, 3), (2, 2), (2, 3))):
            ins4[ki].append(lambda e=e, sc=sc: emit_qk_halfB(e, sc))
        cg0 = [(st, ec) for st in range(4) for ec in range(2)]
        ins4[5] = ins4[5] + [
            lambda st=st, ec=ec: emit_c_group(st, ec, OT_TAGS[((st + ec) % 2, 0)])
            for st, ec in cg0[0:2]]
        ins4[6] = [
            lambda st=st, ec=ec: emit_c_group(st, ec, OT_TAGS[((st + ec) % 2, 0)])
            for st, ec in cg0[2:5]]
        ins4[7] = [
            lambda st=st, ec=ec: emit_c_group(st, ec, OT_TAGS[((st + ec) % 2, 0)])
            for st, ec in cg0[5:8]]
        c4 = emit_section(1, 0, 8, ot10, ins4, carry_in=c2)
        c4()

        # C(qh0, jq1) while its pb tags are free
        emit_c_chunk(0, 1)

        # 5) attention (hp=0, qh=1) with V st 14,15 and remaining K/Q
        #    pair-1 groups interleaved; fillers keep the PE dense
        ot01 = emit_ot_alloc(1)
        ins5 = {
            0: [lambda: emit_v_group(14)],
            1: [lambda: emit_v_group(15)],
            2: [lambda: emit_qk_single(1, 2)],
            3: [lambda: emit_qk_single(1, 3)],
            4: [lambda: emit_qk_single(3, 2)],
            5: [lambda: emit_qk_single(3, 3)],
        }
        for ki in range(6, 16):
            ins5[ki] = [lambda: emit_filler(ot01)]
        c5 = emit_section(0, 1, 16, ot01, ins5)

        # 6) attention (hp=1, qh=1); C(qh1, jq0) groups in the tail slots
        #    (their pb tags free after the slot-11/12 normalizes)
        ot11 = emit_ot_alloc(1)
        ins6 = {}
        for ki in range(1, 13):
            ins6[ki] = [lambda: emit_filler(ot11)]
        cg = [(st, ec) for st in range(8, 12) for ec in range(2)]
        for i, ki in enumerate((13, 14, 15)):
            part = cg[i * 3:(i + 1) * 3] if i < 2 else cg[6:]
            ins6[ki] = [lambda: emit_filler(ot11)] + [
                lambda st=st, ec=ec: emit_c_group(
                    st, ec, OT_TAGS[(0, 0)] if (st + ec) % 2 == 0
                    else OT_TAGS[(1, 0)])
                for st, ec in part]
        c6 = emit_section(1, 1, 16, ot11, ins6, carry_in=c5)
        c6()
        emit_c_chunk(1, 1)

    nc.compile()
    return nc


def _get_program():
    global _NC
    if _NC is None:
        _NC = _build_program()
    return _NC


def kernel(x, w_qkv, b_qkv, w_out, b_out):
    import ml_dtypes
    from concourse.bass_utils import run_bass_kernel_spmd

    BF = ml_dtypes.bfloat16
    x = np.asarray(x, dtype=np.float32)
    w_qkv = np.asarray(w_qkv, dtype=np.float32)
    b_qkv = np.asarray(b_qkv, dtype=np.float32)
    w_out = np.asarray(w_out, dtype=np.float32)
    b_out = np.asarray(b_out, dtype=np.float32)

    nc = _get_program()

    in_maps = []
    for c in range(N_CORES):
        b = c // 4
        g = c % 4
        hs = slice(g * EL, (g + 1) * EL)
        wq = w_qkv[0 * D:1 * D][hs]          # [256, 1024]
        wk = w_qkv[1 * D:2 * D][hs]
        wv = w_qkv[2 * D:3 * D][hs]
        bq = b_qkv[0 * D:1 * D][hs]
        bk = b_qkv[1 * D:2 * D][hs]
        bv = b_qkv[2 * D:3 * D][hs]
        bqk = np.concatenate([bq, bk])       # [512]
        wvx = np.zeros((D, HPC * 65), dtype=np.float32)
        bvx = np.zeros((1, HPC * 65), dtype=np.float32)
        for h in range(HPC):
            wvx[:, 65 * h:65 * h + 64] = wv[h * DH:(h + 1) * DH].T
            bvx[0, 65 * h:65 * h + 64] = bv[h * DH:(h + 1) * DH]
            bvx[0, 65 * h + 64] = 1.0
        in_maps.append({
            "xt": np.ascontiguousarray(x[b].T).astype(BF),             # [1024, 2048]
            "wqk": np.ascontiguousarray(np.concatenate([wq, wk]).T).astype(BF),
            "wv": wvx.astype(BF),                                      # [1024, 260]
            "bqk": np.ascontiguousarray(bqk.reshape(4, 128).T),        # [128, 4] f32
            "bv": bvx.astype(BF),                                      # [1, 260]
            "ones": np.ones((1, 512), dtype=BF),
            "wo": np.ascontiguousarray(w_out[:, hs].T).astype(BF),     # [256, 1024]
        })

    global _last_in_maps
    _last_in_maps = in_maps
    res = run_bass_kernel_spmd(nc, in_maps, list(range(N_CORES)))

    out = np.empty((B, S, D), dtype=np.float32)
    for b in range(B):
        acc = res.results[4 * b]["out"].astype(np.float32)
        for j in range(1, 4):
            acc = acc + res.results[4 * b + j]["out"].astype(np.float32)
        out[b] = acc + b_out[None, :]
    return out


# revision 48
# speedup vs baseline: 1.1039x; 1.0397x over previous
"""Causal self-attention (B=2, S=2048, D=1024, H=16, Dh=64) on 8 NeuronCores.

Sharding: core c -> batch b = c//4, head-group g = c%4 (heads 4g..4g+3).
Each core computes QKV projection for its 4 heads, causal attention
(scores kept transposed: [k, q] layout so no on-chip transposes are
needed), and a partial output projection over its local head dims.
Host sums the 4 partials per batch and adds b_out.

v7: bf16 operands (fp32 PSUM accumulation), software-pipelined attention
units — each unit's second-head attn@V is deferred one slot so the
in-order PE queue always has the next unit's scores to run while the
scalar engine computes exp — plus DMA striped across both HWDGE queues
in need order, split-contraction first projection groups, V/projection/
output-projection work interleaved into the attention windows, and the
output DMA spread across the kernel.
"""

import numpy as np
from contextlib import ExitStack

B = 2
S = 2048
D = 1024
NH = 16
DH = 64
N_CORES = 8
HPC = 4            # heads per core
EL = HPC * DH      # 256 local head dims per core
KD = D // 128      # 8 contraction chunks for projections
KT = S // 128      # 16 key tiles

_NC = None
_last_in_maps = None


def _build_program():
    import concourse.mybir as mybir
    import concourse.tile as tile
    from concourse import bacc

    F32 = mybir.dt.float32
    BF16 = mybir.dt.bfloat16
    Exp = mybir.ActivationFunctionType.Exp

    nc = bacc.Bacc("TRN2", target_bir_lowering=False, debug=False,
                   num_devices=N_CORES)

    xt_d = nc.dram_tensor("xt", [D, S], BF16, kind="ExternalInput")
    wqk_d = nc.dram_tensor("wqk", [D, 2 * EL], BF16, kind="ExternalInput")
    wv_d = nc.dram_tensor("wv", [D, HPC * 65], BF16, kind="ExternalInput")
    bqk_d = nc.dram_tensor("bqk", [128, 4], F32, kind="ExternalInput")
    ones_d = nc.dram_tensor("ones", [1, 512], BF16, kind="ExternalInput")
    bv_d = nc.dram_tensor("bv", [1, HPC * 65], BF16, kind="ExternalInput")
    wo_d = nc.dram_tensor("wo", [EL, D], BF16, kind="ExternalInput")
    out_d = nc.dram_tensor("out", [S, D], BF16, kind="ExternalOutput")

    with nc.allow_low_precision(reason="bf16 matmul operands, fp32 accum"), \
         tile.TileContext(nc) as tc, ExitStack() as ctx:
        const = ctx.enter_context(tc.tile_pool(name="const", bufs=1))
        work = ctx.enter_context(tc.tile_pool(name="work", bufs=1))
        pin = ctx.enter_context(tc.tile_pool(name="pin", bufs=1))
        psS = ctx.enter_context(tc.tile_pool(name="psS", bufs=1, space="PSUM"))
        pb = ctx.enter_context(tc.tile_pool(name="pb", bufs=1, space="PSUM"))
        pp = ctx.enter_context(tc.tile_pool(name="pp", bufs=4))
        sm = ctx.enter_context(tc.tile_pool(name="sm", bufs=2))
        tn = ctx.enter_context(tc.tile_pool(name="tn", bufs=3))
        hb = ctx.enter_context(tc.tile_pool(name="hb", bufs=1))
        ob = ctx.enter_context(tc.tile_pool(name="ob", bufs=4))

        # ---------------- input DMAs ----------------
        # Striped across both HWDGE queues in compute-need order: the
        # first-half projection groups need xt[0..3]+wqk[0..3] first.
        xt_s = [pin.tile([128, S], BF16, tag=f"xt{k}", name=f"xt{k}") for k in range(KD)]
        wqk_s = [pin.tile([128, 2 * EL], BF16, tag=f"wqk{k}", name=f"wqk{k}") for k in range(KD)]
        wv_s = [pin.tile([128, HPC * 65], BF16, tag=f"wv{k}", name=f"wv{k}") for k in range(KD)]
        for k in range(0, KD, 2):
            nc.sync.dma_start(out=xt_s[k], in_=xt_d[128 * k:128 * (k + 1), :])
        for k in range(4):
            nc.scalar.dma_start(out=wqk_s[k], in_=wqk_d[128 * k:128 * (k + 1), :])
        for k in range(1, 4, 2):
            nc.scalar.dma_start(out=xt_s[k], in_=xt_d[128 * k:128 * (k + 1), :])
        for k in range(4, KD):
            nc.scalar.dma_start(out=wqk_s[k], in_=wqk_d[128 * k:128 * (k + 1), :])
        for k in range(5, KD, 2):
            nc.scalar.dma_start(out=xt_s[k], in_=xt_d[128 * k:128 * (k + 1), :])
        for k in range(KD):
            nc.scalar.dma_start(out=wv_s[k], in_=wv_d[128 * k:128 * (k + 1), :])
        wo_s = [const.tile([128, D], BF16, tag=f"wo{i}", name=f"wo{i}") for i in range(2)]
        for i in range(2):
            nc.scalar.dma_start(out=wo_s[i], in_=wo_d[128 * i:128 * (i + 1), :])
        bqk_s = const.tile([128, 4], F32, tag="bqk", name="bqk")
        nc.sync.dma_start(out=bqk_s, in_=bqk_d[:, :])
        bv_s = const.tile([1, HPC * 65], BF16, tag="bv", name="bv")
        nc.sync.dma_start(out=bv_s, in_=bv_d[:, :])
        ones_s = const.tile([1, 512], BF16, tag="ones", name="ones")
        nc.sync.dma_start(out=ones_s, in_=ones_d[:, :])
        # ones living on partition 64, for the denominator broadcast matmul
        ones64_s = const.tile([65, 64], BF16, tag="ones64", name="ones64")
        nc.sync.dma_start(out=ones64_s[64:65, :], in_=ones_d[0:1, 0:64])
        # zero weights for harmless PE keep-warm filler matmuls
        zeros_s = const.tile([128, 65], BF16, tag="zeros", name="zeros")
        nc.gpsimd.memset(zeros_s[:, :], 0.0)

        # ---------------- persistent SBUF tensors ----------------
        # qkT: e-tiles 0,1 = Q (head pairs 0,1), 2,3 = K
        qk_s = [work.tile([128, S], BF16, tag=f"qk{e}", name=f"qk{e}") for e in range(4)]
        # V augmented: per key-tile [128, 4*65]; col 64 of each head = 1.0
        vaug_s = [work.tile([128, HPC * 65], BF16, tag=f"va{t}", name=f"va{t}") for t in range(KT)]
        # normalized attn output, transposed: [d_local, s]
        attnT_s = [work.tile([128, S], BF16, tag=f"at{d}", name=f"at{d}") for d in range(2)]

        # ---------------- emission helpers ----------------
        nps = [0]

        def stag():
            nps[0] += 1
            return f"s{nps[0] % 2}"

        h_sb = {}

        def emit_qk_halfA(e, sc):
            """First half of the contraction (k 0..3) + bias, parked in SBUF."""
            ps = psS.tile([128, 512], F32, tag=stag(), name="psA")
            for k in range(4):
                nc.tensor.matmul(
                    ps,
                    lhsT=wqk_s[k][:, 128 * e:128 * (e + 1)],
                    rhs=xt_s[k][:, 512 * sc:512 * (sc + 1)],
                    start=(k == 0), stop=(k == 3))
            h = hb.tile([128, 512], F32, tag=f"h{e}{sc}", name=f"h{e}{sc}")
            nc.vector.tensor_scalar_add(out=h, in0=ps,
                                        scalar1=bqk_s[:, e:e + 1])
            h_sb[(e, sc)] = h

        def emit_qk_halfB(e, sc):
            """Second half (k 4..7); combined with the parked first half."""
            ps = psS.tile([128, 512], F32, tag=stag(), name="psB")
            for k in range(4, KD):
                nc.tensor.matmul(
                    ps,
                    lhsT=wqk_s[k][:, 128 * e:128 * (e + 1)],
                    rhs=xt_s[k][:, 512 * sc:512 * (sc + 1)],
                    start=(k == 4), stop=(k == KD - 1))
            nc.vector.tensor_add(
                out=qk_s[e][:, 512 * sc:512 * (sc + 1)],
                in0=ps, in1=h_sb[(e, sc)])

        def emit_qk_single(e, sc):
            ps = psS.tile([128, 512], F32, tag=stag(), name="psQ")
            for k in range(KD):
                nc.tensor.matmul(
                    ps,
                    lhsT=wqk_s[k][:, 128 * e:128 * (e + 1)],
                    rhs=xt_s[k][:, 512 * sc:512 * (sc + 1)],
                    start=(k == 0), stop=(k == KD - 1))
            nc.vector.tensor_scalar_add(
                out=qk_s[e][:, 512 * sc:512 * (sc + 1)], in0=ps,
                scalar1=bqk_s[:, e:e + 1])

        def emit_v_group(st):
            ps = psS.tile([128, HPC * 65], F32, tag=stag(), name="psV")
            for k in range(KD):
                nc.tensor.matmul(
                    ps,
                    lhsT=xt_s[k][:, 128 * st:128 * (st + 1)],
                    rhs=wv_s[k],
                    start=(k == 0), stop=False)
            nc.tensor.matmul(ps, lhsT=ones_s[0:1, 0:128], rhs=bv_s,
                             start=False, stop=True)
            nc.vector.tensor_copy(vaug_s[st], ps)

        # attention state per (hp, qh): dict (h2, jq) -> psum tile
        OT_TAGS = {(0, 0): "o00", (1, 0): "o10", (0, 1): "o01", (1, 1): "o11"}

        def emit_ot_alloc(qh):
            return {
                (h2, jq): pb.tile([65, 512], F32, tag=OT_TAGS[(h2, jq)],
                                  name=f"ot{h2}{jq}")
                for h2 in range(2) for jq in range(2)
            }

        def emit_normalize_cols(hp, h2, t, jq_g, c0, c1):
            """Normalize ot columns [c0:c1) into attnT."""
            w = c1 - c0
            den = sm.tile([65, 512], BF16, tag="den", name="den")
            nc.vector.tensor_copy(den[64:65, 0:w], t[64:65, c0:c1])
            rb_ps = psS.tile([64, 512], F32, tag=stag(), name="rb")
            nc.tensor.matmul(rb_ps[:, 0:w], lhsT=ones64_s[64:65, :],
                             rhs=den[64:65, 0:w], start=True, stop=True)
            rb_sb = sm.tile([64, 512], F32, tag="rbs", name="rbs")
            nc.vector.reciprocal_approx_fast(out=rb_sb[:, 0:w],
                                             in_=rb_ps[:, 0:w])
            a0 = 512 * jq_g + c0
            if h2 == 0:
                nc.vector.tensor_mul(
                    out=attnT_s[hp][0:64, a0:a0 + w],
                    in0=t[0:64, c0:c1], in1=rb_sb[:, 0:w])
            else:
                t_n = tn.tile([64, 512], BF16, tag="tn", name="tn")
                nc.vector.tensor_mul(out=t_n[:, 0:w], in0=t[0:64, c0:c1],
                                     in1=rb_sb[:, 0:w])
                nc.sync.dma_start(
                    out=attnT_s[hp][64:128, a0:a0 + w],
                    in_=t_n[:, 0:w])

        def emit_normalize(hp, qh, h2, jq, ot):
            emit_normalize_cols(hp, h2, ot[(h2, jq)], 2 * qh + jq, 0, 512)

        def emit_scores_pair(hp, qh, ki):
            """Scores for both heads of the pair, interleaved h0/h1 per
            q-chunk so the 64-row matmuls overlap on row groups 0/64."""
            qt = qk_s[hp]
            kt_ = qk_s[2 + hp]
            s_ps = [psS.tile([128, 1024], F32, tag=f"s{h2}", name=f"s{h2}")
                    for h2 in range(2)]
            for qq in range(2):
                q0 = 1024 * qh + 512 * qq
                if q0 + 512 <= 128 * ki:
                    continue
                for h2 in range(2):
                    pbase = 64 * h2
                    nc.tensor.matmul(
                        s_ps[h2][:, 512 * qq:512 * (qq + 1)],
                        lhsT=kt_[pbase:pbase + 64, 128 * ki:128 * (ki + 1)],
                        rhs=qt[pbase:pbase + 64, q0:q0 + 512],
                        start=True, stop=True)
            return s_ps

        def emit_exp(qh, ki, s_ps):
            off = max(0, 128 * ki - 1024 * qh)
            p_t = pp.tile([128, 1024], BF16, tag="p", name="pt")
            nc.scalar.activation(
                out=p_t[:, off:1024], in_=s_ps[:, off:1024],
                func=Exp, scale=0.125)
            if 128 * ki >= 1024 * qh:
                dof = 128 * ki - 1024 * qh
                nc.gpsimd.affine_select(
                    out=p_t[:, dof:dof + 128],
                    in_=p_t[:, dof:dof + 128],
                    compare_op=mybir.AluOpType.is_ge, fill=0.0,
                    base=0, pattern=[[1, 128]], channel_multiplier=-1)
            return p_t

        def emit_attnv(hp, qh, ki, h2, p_t, ot):
            h = 2 * hp + h2
            for jq in range(2):
                jq_g = 2 * qh + jq
                if ki > 4 * jq_g + 3:
                    continue
                q0g = max(512 * jq_g, 128 * ki)
                nc.tensor.matmul(
                    ot[(h2, jq)][:, q0g - 512 * jq_g:512],
                    lhsT=vaug_s[ki][:, 65 * h:65 * h + 65],
                    rhs=p_t[:, q0g - 1024 * qh:
                                512 * (jq_g + 1) - 1024 * qh],
                    start=(ki == 0), stop=(ki == 4 * jq_g + 3),
                    skip_group_check=True)
            for jq in range(2):
                if ki == 4 * (2 * qh + jq) + 3:
                    emit_normalize(hp, qh, h2, jq, ot)

        def emit_section(hp, qh, nki, ot, inserts, carry_in=None):
            """Software-pipelined attention section: the h2=1 attn@V of
            each unit is deferred one slot so the PE has the next unit's
            scores available while ACT runs exp. The final deferred unit
            is returned as a closure so the NEXT section can run it after
            its own first scores (cross-section pipelining)."""
            pend = None
            for ki in range(nki):
                ins = inserts.get(ki)
                if ins:
                    for f in ins:
                        f()
                s01 = emit_scores_pair(hp, qh, ki)
                if ki == 0 and carry_in is not None:
                    carry_in()
                if pend is not None:
                    emit_attnv(hp, qh, pend[0], 1, pend[1], ot)
                p0 = emit_exp(qh, ki, s01[0])
                emit_attnv(hp, qh, ki, 0, p0, ot)
                p1 = emit_exp(qh, ki, s01[1])
                pend = (ki, p1)
            last = pend
            return lambda: emit_attnv(hp, qh, last[0], 1, last[1], ot)

        def emit_filler(ot):
            """Zero-weight accumulates into the live jq=1 groups: keep the
            PE issue stream dense (HAM stays warm) without changing math."""
            for h2 in range(2):
                nc.tensor.matmul(
                    ot[(h2, 1)][:, 0:512],
                    lhsT=zeros_s[:, 0:65],
                    rhs=qk_s[h2][:, 0:512],
                    start=False, stop=False,
                    skip_group_check=True)

        def emit_c_group(st, ec, tag):
            ps = pb.tile([128, 512], F32, tag=tag, name="psC")
            for dl in range(2):
                nc.tensor.matmul(
                    ps,
                    lhsT=attnT_s[dl][:, 128 * st:128 * (st + 1)],
                    rhs=wo_s[dl][:, 512 * ec:512 * (ec + 1)],
                    start=(dl == 0), stop=(dl == 1))
            o_t = ob.tile([128, 512], BF16, tag="ob", name="ob")
            nc.vector.tensor_copy(o_t, ps)
            nc.sync.dma_start(
                out=out_d[128 * st:128 * (st + 1),
                          512 * ec:512 * (ec + 1)],
                in_=o_t)

        def emit_c_chunk(qh, jq):
            """Output projection for the 4 seq tiles of chunk jq_g=2qh+jq."""
            jq_g = 2 * qh + jq
            tags = [OT_TAGS[(0, jq)], OT_TAGS[(1, jq)]]
            i = 0
            for st in range(4 * jq_g, 4 * jq_g + 4):
                for ec in range(2):
                    emit_c_group(st, ec, tags[i % 2])
                    i += 1

        # ---------------- emission schedule ----------------
        # 1) K/Q head pair 0: all first-half groups (need only xt[0..3]),
        #    then all second-half groups.
        for e in (2, 0):
            for sc in range(4):
                emit_qk_halfA(e, sc)
        # only the groups attention needs first; sc 2,3 (used by the
        # qh=1 sections only) are deferred into section 4's slots
        for e, sc in ((2, 0), (2, 1), (0, 0), (0, 1)):
            emit_qk_halfB(e, sc)

        # 2) attention (hp=0, qh=0) with V st 0..7 and the head-pair-1
        #    K/Q groups needed early by section 4 interleaved
        ot00 = emit_ot_alloc(0)
        ins2 = {ki: [lambda st=ki: emit_v_group(st)] for ki in range(8)}
        ins2[4].append(lambda: emit_qk_single(3, 0))
        ins2[5].append(lambda: emit_qk_single(1, 0))
        ins2[6].append(lambda: emit_qk_single(1, 1))
        ins2[7].append(lambda: emit_qk_single(3, 1))
        c2 = emit_section(0, 0, 8, ot00, ins2)

        # 4) attention (hp=1, qh=0) with V st 8..13 interleaved and
        #    C(qh0, jq0) in the tail slots (tags free after slot-3/4
        #    normalizes)
        ot10 = emit_ot_alloc(0)
        ins4 = {ki: [lambda st=8 + ki: emit_v_group(st)] for ki in range(6)}
        for ki, (e, sc) in enumerate(((0, 2), (0, 3), (2, 2), (2, 3))):
            ins4[ki].append(lambda e=e, sc=sc: emit_qk_halfB(e, sc))
        cg0 = [(st, ec) for st in range(4) for ec in range(2)]
        ins4[5] = ins4[5] + [
            lambda st=st, ec=ec: emit_c_group(st, ec, OT_TAGS[((st + ec) % 2, 0)])
            for st, ec in cg0[0:2]]
        ins4[6] = [
            lambda st=st, ec=ec: emit_c_group(st, ec, OT_TAGS[((st + ec) % 2, 0)])
            for st, ec in cg0[2:5]]
        ins4[7] = [
            lambda st=st, ec=ec: emit_c_group(st, ec, OT_TAGS[((st + ec) % 2, 0)])
            for st, ec in cg0[5:8]]
        c4 = emit_section(1, 0, 8, ot10, ins4, carry_in=c2)
        c4()

        # C(qh0, jq1) while its pb tags are free
        emit_c_chunk(0, 1)

        # 5) attention (hp=0, qh=1) with V st 14,15 and remaining K/Q
        #    pair-1 groups interleaved; fillers keep the PE dense
        ot01 = emit_ot_alloc(1)
        ins5 = {
            0: [lambda: emit_v_group(14)],
            1: [lambda: emit_v_group(15)],
            2: [lambda: emit_qk_single(1, 2)],
            3: [lambda: emit_qk_single(1, 3)],
            4: [lambda: emit_qk_single(3, 2)],
            5: [lambda: emit_qk_single(3, 3)],
        }
        for ki in range(6, 16):
            ins5[ki] = [lambda: emit_filler(ot01)]
        for ki in range(12, 16):
            # scores and exp narrow here (causal tail): extra filler keeps
            # the PE busy-fraction above the HAM throttle threshold
            ins5[ki].append(lambda: emit_filler(ot01))
        c5 = emit_section(0, 1, 16, ot01, ins5)

        # 6) attention (hp=1, qh=1); C(qh1, jq0) groups in the tail slots
        #    (their pb tags free after the slot-11/12 normalizes)
        ot11 = emit_ot_alloc(1)
        ins6 = {}
        for ki in range(1, 13):
            ins6[ki] = [lambda: emit_filler(ot11)]
        ins6[12].append(lambda: emit_filler(ot11))
        cg = [(st, ec) for st in range(8, 12) for ec in range(2)]
        for i, ki in enumerate((13, 14, 15)):
            part = cg[i * 3:(i + 1) * 3] if i < 2 else cg[6:]
            ins6[ki] = [lambda: emit_filler(ot11)] + [
                lambda st=st, ec=ec: emit_c_group(
                    st, ec, OT_TAGS[(0, 0)] if (st + ec) % 2 == 0
                    else OT_TAGS[(1, 0)])
                for st, ec in part]
        c6 = emit_section(1, 1, 16, ot11, ins6, carry_in=c5)
        c6()
        emit_c_chunk(1, 1)

    nc.compile()
    return nc


def _get_program():
    global _NC
    if _NC is None:
        _NC = _build_program()
    return _NC


def kernel(x, w_qkv, b_qkv, w_out, b_out):
    import ml_dtypes
    from concourse.bass_utils import run_bass_kernel_spmd

    BF = ml_dtypes.bfloat16
    x = np.asarray(x, dtype=np.float32)
    w_qkv = np.asarray(w_qkv, dtype=np.float32)
    b_qkv = np.asarray(b_qkv, dtype=np.float32)
    w_out = np.asarray(w_out, dtype=np.float32)
    b_out = np.asarray(b_out, dtype=np.float32)

    nc = _get_program()

    in_maps = []
    for c in range(N_CORES):
        b = c // 4
        g = c % 4
        hs = slice(g * EL, (g + 1) * EL)
        wq = w_qkv[0 * D:1 * D][hs]          # [256, 1024]
        wk = w_qkv[1 * D:2 * D][hs]
        wv = w_qkv[2 * D:3 * D][hs]
        bq = b_qkv[0 * D:1 * D][hs]
        bk = b_qkv[1 * D:2 * D][hs]
        bv = b_qkv[2 * D:3 * D][hs]
        bqk = np.concatenate([bq, bk])       # [512]
        wvx = np.zeros((D, HPC * 65), dtype=np.float32)
        bvx = np.zeros((1, HPC * 65), dtype=np.float32)
        for h in range(HPC):
            wvx[:, 65 * h:65 * h + 64] = wv[h * DH:(h + 1) * DH].T
            bvx[0, 65 * h:65 * h + 64] = bv[h * DH:(h + 1) * DH]
            bvx[0, 65 * h + 64] = 1.0
        in_maps.append({
            "xt": np.ascontiguousarray(x[b].T).astype(BF),             # [1024, 2048]
            "wqk": np.ascontiguousarray(np.concatenate([wq, wk]).T).astype(BF),
            "wv": wvx.astype(BF),                                      # [1024, 260]
            "bqk": np.ascontiguousarray(bqk.reshape(4, 128).T),        # [128, 4] f32
            "bv": bvx.astype(BF),                                      # [1, 260]
            "ones": np.ones((1, 512), dtype=BF),
            "wo": np.ascontiguousarray(w_out[:, hs].T).astype(BF),     # [256, 1024]
        })

    global _last_in_maps
    _last_in_maps = in_maps
    res = run_bass_kernel_spmd(nc, in_maps, list(range(N_CORES)))

    out = np.empty((B, S, D), dtype=np.float32)
    for b in range(B):
        acc = res.results[4 * b]["out"].astype(np.float32)
        for j in range(1, 4):
            acc = acc + res.results[4 * b + j]["out"].astype(np.float32)
        out[b] = acc + b_out[None, :]
    return out
